# revision 99
# baseline (speedup 1.0000x reference)
"""EvaAttention TRN2 kernel v2: data-parallel over batch across 8 NeuronCores.

Per core (2 batches): bf16 qkv matmuls, joint q/k per-head layernorm stats,
RoPE via folded cos/sin tables, attention with no-max softmax where exp()
tiles are [128,1024] (pair-wide) and PV is computed in flipped orientation
(stationary = probabilities, moving = V) so the output lands as [q, d] in
PSUM -- no transpose and no DRAM round-trip for the attention output.
scale_norm + proj read the SBUF-resident attention output.
"""
import os
import sys

for _p in (
    "/root/.axon_site",
    "/root/.axon_site/_ro/trn_rl_repo",
    "/root/.axon_site/_ro/pypackages",
    "/opt/trn_rl_repo",
    "/opt/pypackages",
):
    if os.path.isdir(_p) and _p not in sys.path:
        sys.path.append(_p)

import numpy as np
import ml_dtypes

import concourse.bass as bass
import concourse.bacc as bacc
import concourse.tile as tile
from concourse import mybir
from concourse.bass_utils import run_bass_kernel_spmd

F32 = mybir.dt.float32
F32R = mybir.dt.float32r
BF16 = mybir.dt.bfloat16
F16 = mybir.dt.float16
Act = mybir.ActivationFunctionType
Alu = mybir.AluOpType
X = mybir.AxisListType.X

B, N, C, H, D = 16, 1025, 1024, 16, 64
EPS = 1e-6
SCALE = D ** -0.5
NCORES = 8
BL = B // NCORES          # batches per core
NT = 9                    # token tiles per batch (pad 1025 -> 1152)
NPAD = NT * 128
HH = 2                    # head halves
HPH = H // HH             # heads per half (8)
PAIRS = HPH // 2          # head pairs per half (4)

_CACHE = {}


def _bcast_mid(ap2d, n):
    """[P, F] AP -> [P, n, F] with step-0 middle dim (free-dim broadcast)."""
    return bass.AP(tensor=ap2d.tensor, offset=ap2d.offset,
                   ap=[ap2d.ap[0], [0, n], ap2d.ap[1]])


def _bcast_last(ap2d, n):
    """[P, F] AP -> [P, F, n] with step-0 last dim."""
    return bass.AP(tensor=ap2d.tensor, offset=ap2d.offset,
                   ap=[ap2d.ap[0], ap2d.ap[1], [0, n]])


class _Pools:
    pass


def _build(has_kbias, has_pbias):
    nc = bacc.Bacc("TRN2", target_bir_lowering=False, debug=False,
                   num_devices=NCORES)

    x_in = nc.dram_tensor("x", [BL, NPAD, C], F16,
                          kind="ExternalInput").ap()
    wt = nc.dram_tensor("wt", [C, 3 * C], F16, kind="ExternalInput").ap()
    nbc = 3 * C
    qkvb = nc.dram_tensor("qkvb", [nbc], F16, kind="ExternalInput").ap()
    ropet = nc.dram_tensor("ropet", [2, NPAD, D], F16,
                           kind="ExternalInput").ap()
    pwt = nc.dram_tensor("pwt", [C, C], F16, kind="ExternalInput").ap()
    wsum = nc.dram_tensor("wsum", [C, 2, 16], F16,
                          kind="ExternalInput").ap()
    bsum = nc.dram_tensor("bsum", [2, 16], F32, kind="ExternalInput").ap()
    pbias = nc.dram_tensor("pbias", [C], F32, kind="ExternalInput").ap()
    identh_d = nc.dram_tensor("identh", [128, 128], F16,
                              kind="ExternalInput").ap()
    identb_d = nc.dram_tensor("identb", [128, 128], BF16,
                              kind="ExternalInput").ap()
    onesb_d = nc.dram_tensor("onesb", [1], BF16, kind="ExternalInput").ap()
    y = nc.dram_tensor("y", [BL, N, C], F32, kind="ExternalOutput").ap()

    with tile.TileContext(nc, pool_alloc_mode="queue") as tc:
        import contextlib
        ctx = contextlib.ExitStack()
        with ctx:
            P = _Pools()
            consts = ctx.enter_context(tc.tile_pool(name="consts", bufs=1))
            P.identh = consts.tile([128, 128], F16, name="identh")
            nc.sync.dma_start(out=P.identh, in_=identh_d)
            P.epst = consts.tile([128, 1], F32, name="epst")
            nc.vector.memset(P.epst, EPS)
            # shared rope tables: [0]=cos [1]=sin (kn_g-folded)
            P.rtab = consts.tile([128, 2, NT, D], F16, name="rtab")
            P.onesh = consts.tile([1, 128], F16, name="onesh")
            nc.vector.memset(P.onesh, 1.0)
            P.biasb = consts.tile([128, nbc], F16, name="biasb")
            nc.sync.dma_start(
                out=P.biasb,
                in_=bass.AP(tensor=qkvb.tensor, offset=qkvb.offset,
                            ap=[[0, 128], [1, nbc]]))
            P.onesb = onesb_d
            if has_pbias:
                P.pbb = consts.tile([128, C], F32, name="pbb")
                nc.sync.dma_start(
                    out=P.pbb,
                    in_=bass.AP(tensor=pbias.tensor, offset=pbias.offset,
                                ap=[[0, 128], [1, C]]))
            else:
                P.pbb = None
            P.wsum = consts.tile([128, 8, 2, 16], F16, name="wsum")
            nc.sync.dma_start(
                out=P.wsum,
                in_=wsum.rearrange("(k p) a s -> p k a s", p=128))
            P.bsum = consts.tile([128, 2, 16], F32, name="bsum")
            nc.sync.dma_start(
                out=P.bsum,
                in_=bass.AP(tensor=bsum.tensor, offset=bsum.offset,
                            ap=[[0, 128], [1, 32]]))
            P.pw = [consts.tile([128, 8, 512], F16, name=f"pwr{oc}")
                    for oc in range(2)]

            # PSUM pools: st 2x2banks, pv 2x1, mm512 3x1  (total 8 banks... 9)
            P.pst = ctx.enter_context(
                tc.tile_pool(name="pst", bufs=2, space="PSUM"))
            P.ppv = ctx.enter_context(
                tc.tile_pool(name="ppv", bufs=1, space="PSUM"))
            P.pmm = ctx.enter_context(
                tc.tile_pool(name="pmm", bufs=2, space="PSUM"))

            # SBUF working pools
            P.xtp = ctx.enter_context(tc.tile_pool(name="xtp", bufs=1))
            P.wp = ctx.enter_context(tc.tile_pool(name="wp", bufs=1))
            P.qkt = ctx.enter_context(tc.tile_pool(name="qkt", bufs=2))
            P.prep = ctx.enter_context(tc.tile_pool(name="prep", bufs=1))
            P.stp = ctx.enter_context(tc.tile_pool(name="stp", bufs=2))
            P.ptp = ctx.enter_context(tc.tile_pool(name="ptp", bufs=10))
            P.attnp = ctx.enter_context(tc.tile_pool(name="attnp", bufs=1))
            P.drp = ctx.enter_context(tc.tile_pool(name="drp", bufs=4))
            # (identb input left declared but unused on-chip)
            P.lnp = ctx.enter_context(tc.tile_pool(name="lnp", bufs=1))
            P.outp = ctx.enter_context(tc.tile_pool(name="outp", bufs=1))

            nc._has_kbias = has_kbias

            # emission order staggers norm_proj(b-1) after qkv(b, h0) so
            # the pmm pool rotation doesn't chain the next batch's qkv
            # behind the previous batch's attention.
            pending = None  # (b, attn) awaiting norm_proj
            for b in range(BL):
                xT = _build_xt(nc, P, b, x_in)
                if b == 0:
                    _late_consts(nc, P, ropet, pwt)
                attn = P.attnp.tile([128, NT, C], F16, tag="attn",
                                    name=f"attn{b}")
                nc.vector.memset(attn[:, NT - 1, :], 0.0)
                for hh in range(HH):
                    QT, KT, V = _qkv_half(nc, P, b, hh, wt, xT)
                    if hh == 0 and pending is not None:
                        _norm_proj(nc, P, pending[0], pending[1], y)
                        pending = None
                    _attn_half(nc, P, b, hh, QT, KT, V, attn)
                pending = (b, attn)
            _norm_proj(nc, P, pending[0], pending[1], y)
    nc.compile()
    return nc


def _late_consts(nc, P, ropet, pwt):
    for f in range(2):
        nc.sync.dma_start(
            out=P.rtab[:, f, :, :],
            in_=ropet[f].rearrange("(t p) d -> p t d", p=128))
    for oc in range(2):
        for k in range(8):
            nc.sync.dma_start(
                out=P.pw[oc][:, k, :],
                in_=pwt[k * 128:(k + 1) * 128, oc * 512:(oc + 1) * 512])


def _build_xt(nc, P, b, x_in):
    """Load x[b]^T via xbar-transpose DMA into xT [128c, 8k, NPAD] bf16."""
    xT = P.xtp.tile([128, 8, NPAD], F16, tag="xT", name=f"xT{b}")
    for k in range(8):
        nc.sync.dma_start_transpose(out=xT[:, k, :],
                                    in_=x_in[b, :, k * 128:(k + 1) * 128])
    return xT


def _qkv_half(nc, P, b, hh, wt, xT):
    """qkv matmuls for one head-half + LN + RoPE -> QT/KT (bf16) and V."""
    QT = P.qkt.tile([128, PAIRS, NPAD], F16, tag="QT", name=f"QT{b}_{hh}")
    KT = P.qkt.tile([128, PAIRS, NPAD], F16, tag="KT", name=f"KT{b}_{hh}")
    V = P.qkt.tile([128, NT, HPH, D + 1], BF16, tag="V", name=f"V{b}_{hh}")
    # ones column of V (col D); untouched pad rows are never read
    nc.sync.dma_start(
        out=V[:, :, :, D:D + 1].rearrange("p t h o -> p (t h) o"),
        in_=bass.AP(tensor=P.onesb.tensor, offset=P.onesb.offset,
                    ap=[[0, 128], [0, NT * HPH], [1, 1]]))
    wch = []
    for oc in range(3):
        col0 = hh * 1536 + oc * 512
        w = P.wp.tile([128, 8, 512], F16, tag=f"w{oc}", name=f"w{oc}")
        for k in range(8):
            nc.sync.dma_start(
                out=w[:, k, :],
                in_=wt[k * 128:(k + 1) * 128, col0:col0 + 512])
        wch.append(w)

    sgrp = P.prep.tile([128, 2, NT, 512], F16, tag="sgrp",
                       name=f"sgrp{b}{hh}",
                       padded_shape=None)
    svar = P.stp.tile([128, NT, 16], F32, tag="svar", name="svar")
    half = D // 2
    for t in range(NT):
        raws = []
        s2 = P.stp.tile([128, 2, HPH], F32, tag="s2", name="s2")
        psum16 = P.pmm.tile([128, 16], F32, tag="mm512", name="sumps")
        for k in range(8):
            nc.tensor.matmul(psum16, xT[:, k, t * 128:(t + 1) * 128],
                             P.wsum[:, k, hh, :], start=(k == 0),
                             stop=(k == 7))
        sums = P.stp.tile([128, 2, HPH], F32, tag="sums", name="sums")
        nc.vector.tensor_tensor(out=sums.rearrange("p a h -> p (a h)"),
                                in0=psum16, in1=P.bsum[:, hh, :],
                                op=Alu.add)
        # mu first (from the sums matmul) so the evacuation centers in place
        mu = P.stp.tile([128, 2, HPH], F32, tag="mu", name="mu")
        nc.vector.tensor_scalar(mu.rearrange("p a h -> p (a h)"),
                                sums.rearrange("p a h -> p (a h)"),
                                1.0 / D, None, op0=Alu.mult)
        for oc in range(2):
            ps = P.pmm.tile([128, 512], F32, tag="mm512", name="qkps")
            for k in range(8):
                nc.tensor.matmul(ps, xT[:, k, t * 128:(t + 1) * 128],
                                 wch[oc][:, k, :], start=(k == 0),
                                 stop=False)
            bc0 = hh * 1536 + oc * 512
            nc.tensor.matmul(ps, P.onesh[:, 0:128],
                             P.biasb[0:1, bc0:bc0 + 512], start=False,
                             stop=True)
            cen = P.prep.tile([128, HPH, D], F16, tag=f"raw{oc}",
                              name=f"cen{oc}")
            nc.vector.tensor_tensor(
                out=cen, in0=ps.rearrange("p (h d) -> p h d", h=HPH),
                in1=_bcast_last(mu[:, oc, :], D), op=Alu.subtract)
            cenf = cen.rearrange("p h d -> p (h d)")
            sq = P.prep.tile([128, HPH * D], F16, tag=f"rb{oc}", name="sq")
            nc.vector.tensor_tensor(out=sq, in0=cenf, in1=cenf, op=Alu.mult)
            nc.vector.tensor_reduce(
                s2[:, oc, :], sq.rearrange("p (h d) -> p h d", h=HPH),
                axis=X, op=Alu.add)
            raws.append(cen)
        nc.vector.tensor_scalar(
            svar[:, t, :], s2.rearrange("p a h -> p (a h)"), 1.0 / D, None,
            op0=Alu.mult)
        # ---- RoPE on centered values: s = cen*cos + swap(cen)*sin ----
        for oc in range(2):
            ctab = P.rtab[:, 0, t, :]
            stab = P.rtab[:, 1, t, :]
            t1 = raws[oc]
            ra = P.prep.tile([128, HPH, D], F16, tag=f"ra{oc}", name="ra")
            nc.gpsimd.tensor_tensor(out=ra, in0=t1,
                                    in1=_bcast_mid(ctab, HPH), op=Alu.mult)
            rb = P.prep.tile([128, HPH, D], F16, tag=f"rb{oc}", name="rb")
            nc.vector.tensor_tensor(
                out=rb[:, :, 0:half], in0=t1[:, :, half:D],
                in1=_bcast_mid(stab[:, 0:half], HPH), op=Alu.mult)
            nc.vector.tensor_tensor(
                out=rb[:, :, half:D], in0=t1[:, :, 0:half],
                in1=_bcast_mid(stab[:, half:D], HPH), op=Alu.mult)
            nc.gpsimd.tensor_tensor(
                out=sgrp[:, oc, t, :].rearrange("p (h d) -> p h d", h=HPH),
                in0=ra, in1=rb, op=Alu.add)
        # ---- v ----
        ps = P.pmm.tile([128, 512], F32, tag="mm512", name="vps")
        for k in range(8):
            nc.tensor.matmul(ps, xT[:, k, t * 128:(t + 1) * 128],
                             wch[2][:, k, :], start=(k == 0), stop=(k == 7))
        nc.scalar.copy(out=V[:, t, :, 0:D],
                       in_=ps.rearrange("p (h d) -> p h d", h=HPH))
    # ---- rstd in two chunks so pass 2 / attention start earlier ----
    rstd = P.stp.tile([128, NT, 16], F32, tag="rstd", name="rstd")
    for ci, (t0, t1) in enumerate(((0, 5), (5, NT))):
        sd = P.stp.tile([128, (t1 - t0) * 16], F32, tag=f"sd{ci}",
                        name="sd")
        nc.scalar.activation(
            sd, svar[:, t0:t1, :].rearrange("p t s -> p (t s)"), Act.Sqrt,
            bias=P.epst[:, 0:1])
        nc.vector.reciprocal(
            rstd[:, t0:t1, :].rearrange("p t s -> p (t s)"), sd)
        nc.vector.tensor_scalar(rstd[:, t0:t1, 0:HPH],
                                rstd[:, t0:t1, 0:HPH], SCALE,
                                None, op0=Alu.mult)
        _pass2(nc, P, t0, t1, sgrp, rstd, QT, KT)
    return QT, KT, V


def _pass2(nc, P, t0, t1, sgrp, rstd, QT, KT):
    # ---- pass 2: scale by rstd, transpose into QT/KT ----
    for t in range(t0, t1):
        for oc in range(2):
            rot = P.prep.tile([128, HPH, D], F16, tag=f"ra{oc}",
                              name="rot")
            nc.vector.tensor_tensor(
                out=rot,
                in0=sgrp[:, oc, t, :].rearrange("p (h d) -> p h d", h=HPH),
                in1=_bcast_last(rstd[:, t, oc * HPH:(oc + 1) * HPH], D),
                op=Alu.mult)
            rotf = rot.rearrange("p h d -> p (h d)")
            psg = P.pmm.tile([128, 4, 128], F16, tag="mm512", name="qktr")
            for p in range(PAIRS):
                nc.tensor.transpose(psg[:, p, :],
                                    rotf[:, p * 128:(p + 1) * 128],
                                    P.identh[:])
            dst = (QT if oc == 0 else KT)[:, :, t * 128:(t + 1) * 128]
            nc.vector.tensor_copy(out=dst, in_=psg)


def _attn_half(nc, P, b, hh, QT, KT, V, attn):
    """Attention for 8 heads of one half; PV flipped -> attn [q, d] bf16."""
    for qc in range(2):
        q0 = qc * 512
        for pp in range(PAIRS):
            heads = (2 * pp, 2 * pp + 1)
            pv = [P.ppv.tile([128, 2, 2, D + 1], F32, tag=f"pv{jj}",
                             name=f"pv{jj}") for jj in range(2)]
            pts = [None] * 9
            for kt in range(9):
                if kt < 8:
                    st = P.pst.tile([128, 2, 512], F32, tag="st", name="st")
                    for s in range(2):
                        r = 64 * s
                        nc.tensor.matmul(
                            st[:, s, :],
                            KT[r:r + 64, pp, kt * 128:(kt + 1) * 128],
                            QT[r:r + 64, pp, q0:q0 + 512])
                    pt = P.ptp.tile([128, 2, 512], BF16, tag="pt", name="pt")
                    nc.scalar.activation(pt, st, Act.Exp)
                else:
                    # k straggler (token 1024): [1, 512] rows per head
                    st = P.pst.tile([128, 2, 512], F32, tag="st", name="st8")
                    for s in range(2):
                        r = 64 * s
                        nc.tensor.matmul(
                            st[0:1, s, :], KT[r:r + 64, pp, 1024:1025],
                            QT[r:r + 64, pp, q0:q0 + 512])
                    pt = P.ptp.tile([128, 2, 512], BF16, tag="pt", name="pt8")
                    nc.scalar.activation(pt[0:1, :, :], st[0:1, :, :],
                                         Act.Exp)
                pts[kt] = pt
            for s in range(2):
                hl = heads[s]
                for j in range(4):
                    dstpv = pv[j // 2][:, j % 2, s, :]
                    for kt in range(8):
                        nc.tensor.matmul(
                            dstpv, pts[kt][:, s, j * 128:(j + 1) * 128],
                            V[:, kt, hl, :], start=(kt == 0), stop=False)
                    nc.tensor.matmul(
                        dstpv, pts[8][0:1, s, j * 128:(j + 1) * 128],
                        V[0:1, 8, hl, :], start=False, stop=True)
            # drains: per (j, s): reciprocal of denominator, scale, store
            for jj in range(2):
                for j2 in range(2):
                    j = 2 * jj + j2
                    for s in range(2):
                        hg = hh * HPH + heads[s]
                        rl = P.drp.tile([128, 1], F32, tag="rl", name="rl")
                        nc.vector.reciprocal(rl, pv[jj][:, j2, s, D:D + 1])
                        dst = attn[:, qc * 4 + j, hg * D:(hg + 1) * D]
                        vc0 = hh * 1536 + 1024
                        bv = P.biasb[:, vc0 + heads[s] * D:
                                     vc0 + (heads[s] + 1) * D]
                        nc.vector.scalar_tensor_tensor(
                            out=dst, in0=pv[jj][:, j2, s, 0:D],
                            scalar=rl[:, 0:1], in1=bv,
                            op0=Alu.mult, op1=Alu.add)
        # ---- q straggler: token 1024 (partition 0 of tile 8) ----
        if qc == 1:
            for pp in range(PAIRS):
                heads = (2 * pp, 2 * pp + 1)
                sp1 = P.pst.tile([128, 18], F32, tag="st", name="sp1")
                for s in range(2):
                    r = 64 * s
                    qstr = QT[r:r + 64, pp, 1024:1025]
                    for kt in range(8):
                        nc.tensor.matmul(
                            sp1[:, 9 * s + kt:9 * s + kt + 1],
                            KT[r:r + 64, pp, kt * 128:(kt + 1) * 128], qstr)
                    nc.tensor.matmul(sp1[0:1, 9 * s + 8:9 * s + 9],
                                     KT[r:r + 64, pp, 1024:1025], qstr)
                p1 = P.ptp.tile([128, 18], BF16, tag="p1", name="p1")
                nc.scalar.activation(p1, sp1, Act.Exp)
                pv1 = P.ppv.tile([128, 2, D + 1], F32, tag="pv0",
                                 name="pvstr")
                for s in range(2):
                    hl = heads[s]
                    for kt in range(8):
                        nc.tensor.matmul(
                            pv1[0:1, s, :], p1[:, 9 * s + kt:9 * s + kt + 1],
                            V[:, kt, hl, :], start=(kt == 0), stop=False)
                    nc.tensor.matmul(pv1[0:1, s, :],
                                     p1[0:1, 9 * s + 8:9 * s + 9],
                                     V[0:1, 8, hl, :], start=False, stop=True)
                for s in range(2):
                    hg = hh * HPH + heads[s]
                    rl1 = P.drp.tile([128, 1], F32, tag="rl", name="rl1")
                    nc.vector.reciprocal(rl1[0:1, :], pv1[0:1, s, D:D + 1])
                    vc0 = hh * 1536 + 1024
                    bv = P.biasb[0:1, vc0 + heads[s] * D:
                                 vc0 + (heads[s] + 1) * D]
                    nc.vector.scalar_tensor_tensor(
                        out=attn[0:1, 8, hg * D:(hg + 1) * D],
                        in0=pv1[0:1, s, 0:D], scalar=rl1[0:1, 0:1], in1=bv,
                        op0=Alu.mult, op1=Alu.add)


def _norm_proj(nc, P, b, attn, y):
    """scale_norm over C + proj matmul + output DMA for batch b."""
    svn = P.stp.tile([128, NT, 2], F32, tag="svn", name="svn")
    for t in range(NT):
        bnt = P.stp.tile([128, 2, 6], F32, tag="bnt", name="bnt")
        for g in range(2):
            nc.vector.bn_stats(bnt[:, g, :],
                               attn[:, t, g * 512:(g + 1) * 512])
        nc.vector.bn_aggr(svn[:, t, :], bnt.rearrange("p g s -> p (g s)"))
    sdn = P.stp.tile([128, NT], F32, tag="sdn", name="sdn")
    nc.scalar.activation(sdn, svn[:, :, 1], Act.Sqrt, bias=P.epst[:, 0:1])
    rstdn = P.stp.tile([128, NT], F32, tag="rstdn", name="rstdn")
    nc.vector.reciprocal(rstdn, sdn)
    for t in range(NT):
        ln = P.lnp.tile([128, C], F16, tag="ln", name="ln")
        nc.gpsimd.tensor_scalar(ln, attn[:, t, :], svn[:, t, 0:1],
                                rstdn[:, t:t + 1], op0=Alu.subtract,
                                op1=Alu.mult)
        lnT = P.lnp.tile([128, 8, 128], F16, tag="lnT", name="lnT")
        for g in range(2):
            psg = P.pmm.tile([128, 4, 128], F16, tag="mm512", name="lntr")
            for k in range(4):
                nc.tensor.transpose(psg[:, k, :],
                                    ln[:, (4 * g + k) * 128:
                                       (4 * g + k + 1) * 128],
                                    P.identh[:])
            dst = lnT[:, 4 * g:4 * g + 4, :]
            nc.vector.tensor_copy(out=dst, in_=psg)
        ostage = P.outp.tile([128, C], F32, tag="ostage", name="ostage")
        for oc in range(2):
            ps = P.pmm.tile([128, 512], F32, tag="mm512", name="projps")
            for k in range(8):
                nc.tensor.matmul(ps, lnT[:, k, :], P.pw[oc][:, k, :],
                                 start=(k == 0), stop=(k == 7))
            dst = ostage[:, oc * 512:(oc + 1) * 512]
            if P.pbb is not None:
                ee = nc.vector if oc == 0 else nc.gpsimd
                ee.tensor_tensor(out=dst, in0=ps,
                                 in1=P.pbb[:, oc * 512:(oc + 1) * 512],
                                 op=Alu.add)
            else:
                nc.vector.tensor_copy(out=dst, in_=ps)
        rows = 128 if t < NT - 1 else N - 128 * (NT - 1)
        nc.sync.dma_start(out=y[b, t * 128:t * 128 + rows, :],
                          in_=ostage[:rows, :])


def _host_prep(inputs):
    """Precompute permuted/transposed weights and folded rope tables."""
    perm = np.concatenate([np.arange(0, D, 2), np.arange(1, D, 2)])
    swap = np.concatenate([np.arange(D // 2, D), np.arange(0, D // 2)])

    qkv_w = np.asarray(inputs["qkv_w"], np.float32)
    rope = np.asarray(inputs["rope"], np.float32)
    sin_t, cos_t = rope[:, :D], rope[:, D:]

    # column order: [half][q|k|v][head-in-half][d]  (d permuted for q,k)
    row_order = np.empty(3 * C, np.int64)
    col = 0
    for hh in range(HH):
        for grp in range(3):
            for h in range(hh * HPH, (hh + 1) * HPH):
                base = grp * C + h * D
                idx = base + (perm if grp < 2 else np.arange(D))
                row_order[col:col + D] = idx
                col += D
    wt = np.ascontiguousarray(qkv_w[row_order, :].T)  # [C, 3C]

    qb = np.asarray(inputs["q_bias"], np.float32)
    kb = np.asarray(inputs["k_bias"], np.float32)
    vb = np.asarray(inputs["v_bias"], np.float32)
    full_bias = np.concatenate([qb, kb, vb])
    qkvb3 = full_bias[row_order].astype(np.float32)
    qkvb = qkvb3

    def make_tables(g, scale):
        gp = np.asarray(g, np.float32)[perm]          # g in permuted coords
        cos_p = cos_t[:, perm]                        # [1024, D]
        sin_p = sin_t[:, perm]
        sgn = np.where(np.arange(D) < D // 2, -1.0, 1.0).astype(np.float32)
        cost = np.zeros((NPAD, D), np.float32)
        sint = np.zeros((NPAD, D), np.float32)
        cost[0] = gp * scale
        cost[1:N] = cos_p * gp[None, :] * scale
        sint[1:N] = sin_p * sgn[None, :] * gp[swap][None, :] * scale
        return cost, sint

    assert np.allclose(np.asarray(inputs["qn_g"]),
                       np.asarray(inputs["kn_g"])), \
        "kernel specialized for qn_g == kn_g (shared rope tables)"
    ck, sk = make_tables(inputs["kn_g"], 1.0)
    ropet = np.stack([ck, sk])  # [2, NPAD, D]

    # per-head column sums of wt for q,k of each half: [C, 2(hh), 16]
    wsum = np.zeros((C, 2, 16), np.float32)
    bsum = np.zeros((2, 16), np.float32)
    for hh_ in range(HH):
        for oc_ in range(2):
            for h_ in range(HPH):
                cols = slice(hh_ * 1536 + oc_ * 512 + h_ * D,
                             hh_ * 1536 + oc_ * 512 + (h_ + 1) * D)
                wsum[:, hh_, oc_ * HPH + h_] = wt[:, cols].sum(1)
                bsum[hh_, oc_ * HPH + h_] = qkvb3[cols].sum()

    norm_g = np.asarray(inputs["norm_g"], np.float32)
    norm_b = np.asarray(inputs["norm_b"], np.float32)
    proj_w = np.asarray(inputs["proj_w"], np.float32)
    proj_b = np.asarray(inputs["proj_b"], np.float32)
    pwt = np.ascontiguousarray((proj_w * norm_g[None, :]).T)  # [C, C]
    pbias = (proj_b + norm_b @ proj_w.T).astype(np.float32)

    return wt, qkvb, ropet, pwt, pbias, wsum, bsum


def kernel(**inputs):
    qn_b = np.asarray(inputs["qn_b"], np.float32)
    kn_b = np.asarray(inputs["kn_b"], np.float32)
    assert not qn_b.any() and not kn_b.any(), \
        "kernel specialized for qn_b == kn_b == 0"

    (wt, qkvb, ropet, pwt, pbias, wsum,
     bsum) = _host_prep(inputs)
    has_kbias = bool(np.asarray(inputs["k_bias"]).any())
    has_pbias = bool(pbias.any())

    key = (has_kbias, has_pbias)
    if key not in _CACHE:
        _CACHE[key] = _build(has_kbias, has_pbias)
    nc = _CACHE[key]

    x = np.asarray(inputs["x"], np.float32)
    xp = np.zeros((B, NPAD, C), np.float16)
    xp[:, :N] = x.astype(np.float16)
    in_maps = []
    for c in range(NCORES):
        in_maps.append({
            "x": np.ascontiguousarray(xp[c * BL:(c + 1) * BL]),
            "wt": wt.astype(np.float16),
            "qkvb": qkvb.astype(np.float16),
            "ropet": ropet.astype(np.float16),
            "pwt": pwt.astype(np.float16),
            "wsum": wsum.astype(np.float16),
            "bsum": bsum,
            "pbias": pbias,
            "identh": np.eye(128, dtype=np.float16),
            "identb": np.eye(128, dtype=np.float32).astype(ml_dtypes.bfloat16),
            "onesb": np.ones(1, dtype=np.float32).astype(ml_dtypes.bfloat16),
        })
    res = run_bass_kernel_spmd(nc, in_maps, core_ids=list(range(NCORES)))
    out = np.concatenate([res.results[c]["y"] for c in range(NCORES)], axis=0)
    return out.astype(np.float32)



# revision 102
# speedup vs baseline: 1.0014x; 1.0014x over previous
"""EvaAttention TRN2 kernel v2: data-parallel over batch across 8 NeuronCores.

Per core (2 batches): bf16 qkv matmuls, joint q/k per-head layernorm stats,
RoPE via folded cos/sin tables, attention with no-max softmax where exp()
tiles are [128,1024] (pair-wide) and PV is computed in flipped orientation
(stationary = probabilities, moving = V) so the output lands as [q, d] in
PSUM -- no transpose and no DRAM round-trip for the attention output.
scale_norm + proj read the SBUF-resident attention output.
"""
import os
import sys

for _p in (
    "/root/.axon_site",
    "/root/.axon_site/_ro/trn_rl_repo",
    "/root/.axon_site/_ro/pypackages",
    "/opt/trn_rl_repo",
    "/opt/pypackages",
):
    if os.path.isdir(_p) and _p not in sys.path:
        sys.path.append(_p)

import numpy as np
import ml_dtypes

import concourse.bass as bass
import concourse.bacc as bacc
import concourse.tile as tile
from concourse import mybir
from concourse.bass_utils import run_bass_kernel_spmd

F32 = mybir.dt.float32
F32R = mybir.dt.float32r
BF16 = mybir.dt.bfloat16
F16 = mybir.dt.float16
Act = mybir.ActivationFunctionType
Alu = mybir.AluOpType
X = mybir.AxisListType.X

B, N, C, H, D = 16, 1025, 1024, 16, 64
EPS = 1e-6
SCALE = D ** -0.5
NCORES = 8
BL = B // NCORES          # batches per core
NT = 9                    # token tiles per batch (pad 1025 -> 1152)
NPAD = NT * 128
HH = 2                    # head halves
HPH = H // HH             # heads per half (8)
PAIRS = HPH // 2          # head pairs per half (4)

_CACHE = {}


def _bcast_mid(ap2d, n):
    """[P, F] AP -> [P, n, F] with step-0 middle dim (free-dim broadcast)."""
    return bass.AP(tensor=ap2d.tensor, offset=ap2d.offset,
                   ap=[ap2d.ap[0], [0, n], ap2d.ap[1]])


def _bcast_last(ap2d, n):
    """[P, F] AP -> [P, F, n] with step-0 last dim."""
    return bass.AP(tensor=ap2d.tensor, offset=ap2d.offset,
                   ap=[ap2d.ap[0], ap2d.ap[1], [0, n]])


class _Pools:
    pass


def _build(has_kbias, has_pbias):
    nc = bacc.Bacc("TRN2", target_bir_lowering=False, debug=False,
                   num_devices=NCORES)

    x_in = nc.dram_tensor("x", [BL, NPAD, C], F16,
                          kind="ExternalInput").ap()
    wt = nc.dram_tensor("wt", [C, 3 * C], F16, kind="ExternalInput").ap()
    nbc = 3 * C
    qkvb = nc.dram_tensor("qkvb", [nbc], F16, kind="ExternalInput").ap()
    ropet = nc.dram_tensor("ropet", [2, NPAD, D], F16,
                           kind="ExternalInput").ap()
    pwt = nc.dram_tensor("pwt", [C, C], F16, kind="ExternalInput").ap()
    wsum = nc.dram_tensor("wsum", [C, 2, 16], F16,
                          kind="ExternalInput").ap()
    bsum = nc.dram_tensor("bsum", [2, 16], F32, kind="ExternalInput").ap()
    pbias = nc.dram_tensor("pbias", [C], F32, kind="ExternalInput").ap()
    identh_d = nc.dram_tensor("identh", [128, 128], F16,
                              kind="ExternalInput").ap()
    identb_d = nc.dram_tensor("identb", [128, 128], BF16,
                              kind="ExternalInput").ap()
    onesb_d = nc.dram_tensor("onesb", [1], BF16, kind="ExternalInput").ap()
    y = nc.dram_tensor("y", [BL, N, C], F32, kind="ExternalOutput").ap()

    with tile.TileContext(nc, pool_alloc_mode="queue") as tc:
        import contextlib
        ctx = contextlib.ExitStack()
        with ctx:
            P = _Pools()
            consts = ctx.enter_context(tc.tile_pool(name="consts", bufs=1))
            P.identh = consts.tile([128, 128], F16, name="identh")
            nc.sync.dma_start(out=P.identh, in_=identh_d)
            P.epst = consts.tile([128, 1], F32, name="epst")
            nc.vector.memset(P.epst, EPS)
            # shared rope tables: [0]=cos [1]=sin (kn_g-folded)
            P.rtab = consts.tile([128, 2, NT, D], F16, name="rtab")
            P.onesh = consts.tile([1, 128], F16, name="onesh")
            nc.vector.memset(P.onesh, 1.0)
            P.biasb = consts.tile([128, nbc], F16, name="biasb")
            nc.sync.dma_start(
                out=P.biasb,
                in_=bass.AP(tensor=qkvb.tensor, offset=qkvb.offset,
                            ap=[[0, 128], [1, nbc]]))
            P.onesb = onesb_d
            if has_pbias:
                P.pbb = consts.tile([128, C], F32, name="pbb")
                nc.sync.dma_start(
                    out=P.pbb,
                    in_=bass.AP(tensor=pbias.tensor, offset=pbias.offset,
                                ap=[[0, 128], [1, C]]))
            else:
                P.pbb = None
            P.wsum = consts.tile([128, 8, 2, 16], F16, name="wsum")
            nc.sync.dma_start(
                out=P.wsum,
                in_=wsum.rearrange("(k p) a s -> p k a s", p=128))
            P.bsum = consts.tile([128, 2, 16], F32, name="bsum")
            nc.sync.dma_start(
                out=P.bsum,
                in_=bass.AP(tensor=bsum.tensor, offset=bsum.offset,
                            ap=[[0, 128], [1, 32]]))
            P.pw = [consts.tile([128, 8, 512], F16, name=f"pwr{oc}")
                    for oc in range(2)]

            # PSUM pools: st 2x2banks, pv 2x1, mm512 3x1  (total 8 banks... 9)
            P.pst = ctx.enter_context(
                tc.tile_pool(name="pst", bufs=2, space="PSUM"))
            P.ppv = ctx.enter_context(
                tc.tile_pool(name="ppv", bufs=1, space="PSUM"))
            P.pmm = ctx.enter_context(
                tc.tile_pool(name="pmm", bufs=2, space="PSUM"))

            # SBUF working pools
            P.xtp = ctx.enter_context(tc.tile_pool(name="xtp", bufs=1))
            P.wp = ctx.enter_context(tc.tile_pool(name="wp", bufs=1))
            P.qkt = ctx.enter_context(tc.tile_pool(name="qkt", bufs=2))
            P.prep = ctx.enter_context(tc.tile_pool(name="prep", bufs=1))
            P.stp = ctx.enter_context(tc.tile_pool(name="stp", bufs=2))
            P.ptp = ctx.enter_context(tc.tile_pool(name="ptp", bufs=10))
            P.attnp = ctx.enter_context(tc.tile_pool(name="attnp", bufs=1))
            P.drp = ctx.enter_context(tc.tile_pool(name="drp", bufs=4))
            # (identb input left declared but unused on-chip)
            P.lnp = ctx.enter_context(tc.tile_pool(name="lnp", bufs=1))
            P.outp = ctx.enter_context(tc.tile_pool(name="outp", bufs=1))

            nc._has_kbias = has_kbias

            # emission order staggers norm_proj(b-1) after qkv(b, h0) so
            # the pmm pool rotation doesn't chain the next batch's qkv
            # behind the previous batch's attention.
            pending = None  # (b, attn) awaiting norm_proj
            for b in range(BL):
                xT = _build_xt(nc, P, b, x_in)
                if b == 0:
                    _late_consts(nc, P, ropet, pwt)
                attn = P.attnp.tile([128, NT, C], F16, tag="attn",
                                    name=f"attn{b}")
                nc.vector.memset(attn[:, NT - 1, :], 0.0)
                for hh in range(HH):
                    QT, KT, V = _qkv_half(nc, P, b, hh, wt, xT)
                    if hh == 0 and pending is not None:
                        _norm_proj(nc, P, pending[0], pending[1], y)
                        pending = None
                    _attn_half(nc, P, b, hh, QT, KT, V, attn)
                pending = (b, attn)
            _norm_proj(nc, P, pending[0], pending[1], y)
    nc.compile()
    return nc


def _late_consts(nc, P, ropet, pwt):
    for f in range(2):
        nc.sync.dma_start(
            out=P.rtab[:, f, :, :],
            in_=ropet[f].rearrange("(t p) d -> p t d", p=128))
    for oc in range(2):
        nc.sync.dma_start(
            out=P.pw[oc],
            in_=pwt[:, oc * 512:(oc + 1) * 512].rearrange(
                "(k p) o -> p k o", p=128))


def _build_xt(nc, P, b, x_in):
    """Load x[b]^T via xbar-transpose DMA into xT [128c, 8k, NPAD] bf16."""
    xT = P.xtp.tile([128, 8, NPAD], F16, tag="xT", name=f"xT{b}")
    for k in range(8):
        nc.sync.dma_start_transpose(out=xT[:, k, :],
                                    in_=x_in[b, :, k * 128:(k + 1) * 128])
    return xT


def _qkv_half(nc, P, b, hh, wt, xT):
    """qkv matmuls for one head-half + LN + RoPE -> QT/KT (bf16) and V."""
    QT = P.qkt.tile([128, PAIRS, NPAD], F16, tag="QT", name=f"QT{b}_{hh}")
    KT = P.qkt.tile([128, PAIRS, NPAD], F16, tag="KT", name=f"KT{b}_{hh}")
    V = P.qkt.tile([128, NT, HPH, D + 1], BF16, tag="V", name=f"V{b}_{hh}")
    # ones column of V (col D); untouched pad rows are never read
    nc.sync.dma_start(
        out=V[:, :, :, D:D + 1].rearrange("p t h o -> p (t h) o"),
        in_=bass.AP(tensor=P.onesb.tensor, offset=P.onesb.offset,
                    ap=[[0, 128], [0, NT * HPH], [1, 1]]))
    wch = []
    for oc in range(3):
        col0 = hh * 1536 + oc * 512
        w = P.wp.tile([128, 8, 512], F16, tag=f"w{oc}", name=f"w{oc}")
        nc.sync.dma_start(
            out=w, in_=wt[:, col0:col0 + 512].rearrange("(k p) o -> p k o",
                                                        p=128))
        wch.append(w)

    sgrp = P.prep.tile([128, 2, NT, 512], F16, tag="sgrp",
                       name=f"sgrp{b}{hh}",
                       padded_shape=None)
    svar = P.stp.tile([128, NT, 16], F32, tag="svar", name="svar")
    half = D // 2
    for t in range(NT):
        raws = []
        s2 = P.stp.tile([128, 2, HPH], F32, tag="s2", name="s2")
        psum16 = P.pmm.tile([128, 16], F32, tag="mm512", name="sumps")
        for k in range(8):
            nc.tensor.matmul(psum16, xT[:, k, t * 128:(t + 1) * 128],
                             P.wsum[:, k, hh, :], start=(k == 0),
                             stop=(k == 7))
        sums = P.stp.tile([128, 2, HPH], F32, tag="sums", name="sums")
        nc.vector.tensor_tensor(out=sums.rearrange("p a h -> p (a h)"),
                                in0=psum16, in1=P.bsum[:, hh, :],
                                op=Alu.add)
        # mu first (from the sums matmul) so the evacuation centers in place
        mu = P.stp.tile([128, 2, HPH], F32, tag="mu", name="mu")
        nc.vector.tensor_scalar(mu.rearrange("p a h -> p (a h)"),
                                sums.rearrange("p a h -> p (a h)"),
                                1.0 / D, None, op0=Alu.mult)
        for oc in range(2):
            ps = P.pmm.tile([128, 512], F32, tag="mm512", name="qkps")
            for k in range(8):
                nc.tensor.matmul(ps, xT[:, k, t * 128:(t + 1) * 128],
                                 wch[oc][:, k, :], start=(k == 0),
                                 stop=False)
            bc0 = hh * 1536 + oc * 512
            nc.tensor.matmul(ps, P.onesh[:, 0:128],
                             P.biasb[0:1, bc0:bc0 + 512], start=False,
                             stop=True)
            cen = P.prep.tile([128, HPH, D], F16, tag=f"raw{oc}",
                              name=f"cen{oc}")
            nc.vector.tensor_tensor(
                out=cen, in0=ps.rearrange("p (h d) -> p h d", h=HPH),
                in1=_bcast_last(mu[:, oc, :], D), op=Alu.subtract)
            cenf = cen.rearrange("p h d -> p (h d)")
            sq = P.prep.tile([128, HPH * D], F16, tag=f"rb{oc}", name="sq")
            nc.vector.tensor_tensor(out=sq, in0=cenf, in1=cenf, op=Alu.mult)
            nc.vector.tensor_reduce(
                s2[:, oc, :], sq.rearrange("p (h d) -> p h d", h=HPH),
                axis=X, op=Alu.add)
            raws.append(cen)
        nc.vector.tensor_scalar(
            svar[:, t, :], s2.rearrange("p a h -> p (a h)"), 1.0 / D, None,
            op0=Alu.mult)
        # ---- RoPE on centered values: s = cen*cos + swap(cen)*sin ----
        for oc in range(2):
            ctab = P.rtab[:, 0, t, :]
            stab = P.rtab[:, 1, t, :]
            t1 = raws[oc]
            ra = P.prep.tile([128, HPH, D], F16, tag=f"ra{oc}", name="ra")
            nc.gpsimd.tensor_tensor(out=ra, in0=t1,
                                    in1=_bcast_mid(ctab, HPH), op=Alu.mult)
            rb = P.prep.tile([128, HPH, D], F16, tag=f"rb{oc}", name="rb")
            nc.vector.tensor_tensor(
                out=rb[:, :, 0:half], in0=t1[:, :, half:D],
                in1=_bcast_mid(stab[:, 0:half], HPH), op=Alu.mult)
            nc.vector.tensor_tensor(
                out=rb[:, :, half:D], in0=t1[:, :, 0:half],
                in1=_bcast_mid(stab[:, half:D], HPH), op=Alu.mult)
            nc.gpsimd.tensor_tensor(
                out=sgrp[:, oc, t, :].rearrange("p (h d) -> p h d", h=HPH),
                in0=ra, in1=rb, op=Alu.add)
        # ---- v ----
        ps = P.pmm.tile([128, 512], F32, tag="mm512", name="vps")
        for k in range(8):
            nc.tensor.matmul(ps, xT[:, k, t * 128:(t + 1) * 128],
                             wch[2][:, k, :], start=(k == 0), stop=(k == 7))
        nc.scalar.copy(out=V[:, t, :, 0:D],
                       in_=ps.rearrange("p (h d) -> p h d", h=HPH))
    # ---- rstd in two chunks so pass 2 / attention start earlier ----
    rstd = P.stp.tile([128, NT, 16], F32, tag="rstd", name="rstd")
    for ci, (t0, t1) in enumerate(((0, 5), (5, NT))):
        sd = P.stp.tile([128, (t1 - t0) * 16], F32, tag=f"sd{ci}",
                        name="sd")
        nc.scalar.activation(
            sd, svar[:, t0:t1, :].rearrange("p t s -> p (t s)"), Act.Sqrt,
            bias=P.epst[:, 0:1])
        nc.vector.reciprocal(
            rstd[:, t0:t1, :].rearrange("p t s -> p (t s)"), sd)
        nc.vector.tensor_scalar(rstd[:, t0:t1, 0:HPH],
                                rstd[:, t0:t1, 0:HPH], SCALE,
                                None, op0=Alu.mult)
        _pass2(nc, P, t0, t1, sgrp, rstd, QT, KT)
    return QT, KT, V


def _pass2(nc, P, t0, t1, sgrp, rstd, QT, KT):
    # ---- pass 2: scale by rstd, transpose into QT/KT ----
    for t in range(t0, t1):
        for oc in range(2):
            rot = P.prep.tile([128, HPH, D], F16, tag=f"ra{oc}",
                              name="rot")
            nc.vector.tensor_tensor(
                out=rot,
                in0=sgrp[:, oc, t, :].rearrange("p (h d) -> p h d", h=HPH),
                in1=_bcast_last(rstd[:, t, oc * HPH:(oc + 1) * HPH], D),
                op=Alu.mult)
            rotf = rot.rearrange("p h d -> p (h d)")
            psg = P.pmm.tile([128, 4, 128], F16, tag="mm512", name="qktr")
            for p in range(PAIRS):
                nc.tensor.transpose(psg[:, p, :],
                                    rotf[:, p * 128:(p + 1) * 128],
                                    P.identh[:])
            dst = (QT if oc == 0 else KT)[:, :, t * 128:(t + 1) * 128]
            nc.vector.tensor_copy(out=dst, in_=psg)


def _attn_half(nc, P, b, hh, QT, KT, V, attn):
    """Attention for 8 heads of one half; PV flipped -> attn [q, d] bf16."""
    for qc in range(2):
        q0 = qc * 512
        for pp in range(PAIRS):
            heads = (2 * pp, 2 * pp + 1)
            pv = [P.ppv.tile([128, 2, 2, D + 1], F32, tag=f"pv{jj}",
                             name=f"pv{jj}") for jj in range(2)]
            pts = [None] * 9
            for kt in range(9):
                if kt < 8:
                    st = P.pst.tile([128, 2, 512], F32, tag="st", name="st")
                    for s in range(2):
                        r = 64 * s
                        nc.tensor.matmul(
                            st[:, s, :],
                            KT[r:r + 64, pp, kt * 128:(kt + 1) * 128],
                            QT[r:r + 64, pp, q0:q0 + 512])
                    pt = P.ptp.tile([128, 2, 512], BF16, tag="pt", name="pt")
                    nc.scalar.activation(pt, st, Act.Exp)
                else:
                    # k straggler (token 1024): [1, 512] rows per head
                    st = P.pst.tile([128, 2, 512], F32, tag="st", name="st8")
                    for s in range(2):
                        r = 64 * s
                        nc.tensor.matmul(
                            st[0:1, s, :], KT[r:r + 64, pp, 1024:1025],
                            QT[r:r + 64, pp, q0:q0 + 512])
                    pt = P.ptp.tile([128, 2, 512], BF16, tag="pt", name="pt8")
                    nc.scalar.activation(pt[0:1, :, :], st[0:1, :, :],
                                         Act.Exp)
                pts[kt] = pt
            for s in range(2):
                hl = heads[s]
                for j in range(4):
                    dstpv = pv[j // 2][:, j % 2, s, :]
                    for kt in range(8):
                        nc.tensor.matmul(
                            dstpv, pts[kt][:, s, j * 128:(j + 1) * 128],
                            V[:, kt, hl, :], start=(kt == 0), stop=False)
                    nc.tensor.matmul(
                        dstpv, pts[8][0:1, s, j * 128:(j + 1) * 128],
                        V[0:1, 8, hl, :], start=False, stop=True)
            # drains: per (j, s): reciprocal of denominator, scale, store
            for jj in range(2):
                for j2 in range(2):
                    j = 2 * jj + j2
                    for s in range(2):
                        hg = hh * HPH + heads[s]
                        rl = P.drp.tile([128, 1], F32, tag="rl", name="rl")
                        nc.vector.reciprocal(rl, pv[jj][:, j2, s, D:D + 1])
                        dst = attn[:, qc * 4 + j, hg * D:(hg + 1) * D]
                        vc0 = hh * 1536 + 1024
                        bv = P.biasb[:, vc0 + heads[s] * D:
                                     vc0 + (heads[s] + 1) * D]
                        nc.vector.scalar_tensor_tensor(
                            out=dst, in0=pv[jj][:, j2, s, 0:D],
                            scalar=rl[:, 0:1], in1=bv,
                            op0=Alu.mult, op1=Alu.add)
        # ---- q straggler: token 1024 (partition 0 of tile 8) ----
        if qc == 1:
            for pp in range(PAIRS):
                heads = (2 * pp, 2 * pp + 1)
                sp1 = P.pst.tile([128, 18], F32, tag="st", name="sp1")
                for s in range(2):
                    r = 64 * s
                    qstr = QT[r:r + 64, pp, 1024:1025]
                    for kt in range(8):
                        nc.tensor.matmul(
                            sp1[:, 9 * s + kt:9 * s + kt + 1],
                            KT[r:r + 64, pp, kt * 128:(kt + 1) * 128], qstr)
                    nc.tensor.matmul(sp1[0:1, 9 * s + 8:9 * s + 9],
                                     KT[r:r + 64, pp, 1024:1025], qstr)
                p1 = P.ptp.tile([128, 18], BF16, tag="p1", name="p1")
                nc.scalar.activation(p1, sp1, Act.Exp)
                pv1 = P.ppv.tile([128, 2, D + 1], F32, tag="pv0",
                                 name="pvstr")
                for s in range(2):
                    hl = heads[s]
                    for kt in range(8):
                        nc.tensor.matmul(
                            pv1[0:1, s, :], p1[:, 9 * s + kt:9 * s + kt + 1],
                            V[:, kt, hl, :], start=(kt == 0), stop=False)
                    nc.tensor.matmul(pv1[0:1, s, :],
                                     p1[0:1, 9 * s + 8:9 * s + 9],
                                     V[0:1, 8, hl, :], start=False, stop=True)
                for s in range(2):
                    hg = hh * HPH + heads[s]
                    rl1 = P.drp.tile([128, 1], F32, tag="rl", name="rl1")
                    nc.vector.reciprocal(rl1[0:1, :], pv1[0:1, s, D:D + 1])
                    vc0 = hh * 1536 + 1024
                    bv = P.biasb[0:1, vc0 + heads[s] * D:
                                 vc0 + (heads[s] + 1) * D]
                    nc.vector.scalar_tensor_tensor(
                        out=attn[0:1, 8, hg * D:(hg + 1) * D],
                        in0=pv1[0:1, s, 0:D], scalar=rl1[0:1, 0:1], in1=bv,
                        op0=Alu.mult, op1=Alu.add)


def _norm_proj(nc, P, b, attn, y):
    """scale_norm over C + proj matmul + output DMA for batch b."""
    svn = P.stp.tile([128, NT, 2], F32, tag="svn", name="svn")
    for t in range(NT):
        bnt = P.stp.tile([128, 2, 6], F32, tag="bnt", name="bnt")
        for g in range(2):
            nc.vector.bn_stats(bnt[:, g, :],
                               attn[:, t, g * 512:(g + 1) * 512])
        nc.vector.bn_aggr(svn[:, t, :], bnt.rearrange("p g s -> p (g s)"))
    sdn = P.stp.tile([128, NT], F32, tag="sdn", name="sdn")
    nc.scalar.activation(sdn, svn[:, :, 1], Act.Sqrt, bias=P.epst[:, 0:1])
    rstdn = P.stp.tile([128, NT], F32, tag="rstdn", name="rstdn")
    nc.vector.reciprocal(rstdn, sdn)
    for t in range(NT):
        ln = P.lnp.tile([128, C], F16, tag="ln", name="ln")
        nc.gpsimd.tensor_scalar(ln, attn[:, t, :], svn[:, t, 0:1],
                                rstdn[:, t:t + 1], op0=Alu.subtract,
                                op1=Alu.mult)
        lnT = P.lnp.tile([128, 8, 128], F16, tag="lnT", name="lnT")
        for g in range(2):
            psg = P.pmm.tile([128, 4, 128], F16, tag="mm512", name="lntr")
            for k in range(4):
                nc.tensor.transpose(psg[:, k, :],
                                    ln[:, (4 * g + k) * 128:
                                       (4 * g + k + 1) * 128],
                                    P.identh[:])
            dst = lnT[:, 4 * g:4 * g + 4, :]
            nc.vector.tensor_copy(out=dst, in_=psg)
        ostage = P.outp.tile([128, C], F32, tag="ostage", name="ostage")
        for oc in range(2):
            ps = P.pmm.tile([128, 512], F32, tag="mm512", name="projps")
            for k in range(8):
                nc.tensor.matmul(ps, lnT[:, k, :], P.pw[oc][:, k, :],
                                 start=(k == 0), stop=(k == 7))
            dst = ostage[:, oc * 512:(oc + 1) * 512]
            if P.pbb is not None:
                ee = nc.vector if oc == 0 else nc.gpsimd
                ee.tensor_tensor(out=dst, in0=ps,
                                 in1=P.pbb[:, oc * 512:(oc + 1) * 512],
                                 op=Alu.add)
            else:
                nc.vector.tensor_copy(out=dst, in_=ps)
        rows = 128 if t < NT - 1 else N - 128 * (NT - 1)
        nc.sync.dma_start(out=y[b, t * 128:t * 128 + rows, :],
                          in_=ostage[:rows, :])


def _host_prep(inputs):
    """Precompute permuted/transposed weights and folded rope tables."""
    perm = np.concatenate([np.arange(0, D, 2), np.arange(1, D, 2)])
    swap = np.concatenate([np.arange(D // 2, D), np.arange(0, D // 2)])

    qkv_w = np.asarray(inputs["qkv_w"], np.float32)
    rope = np.asarray(inputs["rope"], np.float32)
    sin_t, cos_t = rope[:, :D], rope[:, D:]

    # column order: [half][q|k|v][head-in-half][d]  (d permuted for q,k)
    row_order = np.empty(3 * C, np.int64)
    col = 0
    for hh in range(HH):
        for grp in range(3):
            for h in range(hh * HPH, (hh + 1) * HPH):
                base = grp * C + h * D
                idx = base + (perm if grp < 2 else np.arange(D))
                row_order[col:col + D] = idx
                col += D
    wt = np.ascontiguousarray(qkv_w[row_order, :].T)  # [C, 3C]

    qb = np.asarray(inputs["q_bias"], np.float32)
    kb = np.asarray(inputs["k_bias"], np.float32)
    vb = np.asarray(inputs["v_bias"], np.float32)
    full_bias = np.concatenate([qb, kb, vb])
    qkvb3 = full_bias[row_order].astype(np.float32)
    qkvb = qkvb3

    def make_tables(g, scale):
        gp = np.asarray(g, np.float32)[perm]          # g in permuted coords
        cos_p = cos_t[:, perm]                        # [1024, D]
        sin_p = sin_t[:, perm]
        sgn = np.where(np.arange(D) < D // 2, -1.0, 1.0).astype(np.float32)
        cost = np.zeros((NPAD, D), np.float32)
        sint = np.zeros((NPAD, D), np.float32)
        cost[0] = gp * scale
        cost[1:N] = cos_p * gp[None, :] * scale
        sint[1:N] = sin_p * sgn[None, :] * gp[swap][None, :] * scale
        return cost, sint

    assert np.allclose(np.asarray(inputs["qn_g"]),
                       np.asarray(inputs["kn_g"])), \
        "kernel specialized for qn_g == kn_g (shared rope tables)"
    ck, sk = make_tables(inputs["kn_g"], 1.0)
    ropet = np.stack([ck, sk])  # [2, NPAD, D]

    # per-head column sums of wt for q,k of each half: [C, 2(hh), 16]
    wsum = np.zeros((C, 2, 16), np.float32)
    bsum = np.zeros((2, 16), np.float32)
    for hh_ in range(HH):
        for oc_ in range(2):
            for h_ in range(HPH):
                cols = slice(hh_ * 1536 + oc_ * 512 + h_ * D,
                             hh_ * 1536 + oc_ * 512 + (h_ + 1) * D)
                wsum[:, hh_, oc_ * HPH + h_] = wt[:, cols].sum(1)
                bsum[hh_, oc_ * HPH + h_] = qkvb3[cols].sum()

    norm_g = np.asarray(inputs["norm_g"], np.float32)
    norm_b = np.asarray(inputs["norm_b"], np.float32)
    proj_w = np.asarray(inputs["proj_w"], np.float32)
    proj_b = np.asarray(inputs["proj_b"], np.float32)
    pwt = np.ascontiguousarray((proj_w * norm_g[None, :]).T)  # [C, C]
    pbias = (proj_b + norm_b @ proj_w.T).astype(np.float32)

    return wt, qkvb, ropet, pwt, pbias, wsum, bsum


def kernel(**inputs):
    qn_b = np.asarray(inputs["qn_b"], np.float32)
    kn_b = np.asarray(inputs["kn_b"], np.float32)
    assert not qn_b.any() and not kn_b.any(), \
        "kernel specialized for qn_b == kn_b == 0"

    (wt, qkvb, ropet, pwt, pbias, wsum,
     bsum) = _host_prep(inputs)
    has_kbias = bool(np.asarray(inputs["k_bias"]).any())
    has_pbias = bool(pbias.any())

    key = (has_kbias, has_pbias)
    if key not in _CACHE:
        _CACHE[key] = _build(has_kbias, has_pbias)
    nc = _CACHE[key]

    x = np.asarray(inputs["x"], np.float32)
    xp = np.zeros((B, NPAD, C), np.float16)
    xp[:, :N] = x.astype(np.float16)
    in_maps = []
    for c in range(NCORES):
        in_maps.append({
            "x": np.ascontiguousarray(xp[c * BL:(c + 1) * BL]),
            "wt": wt.astype(np.float16),
            "qkvb": qkvb.astype(np.float16),
            "ropet": ropet.astype(np.float16),
            "pwt": pwt.astype(np.float16),
            "wsum": wsum.astype(np.float16),
            "bsum": bsum,
            "pbias": pbias,
            "identh": np.eye(128, dtype=np.float16),
            "identb": np.eye(128, dtype=np.float32).astype(ml_dtypes.bfloat16),
            "onesb": np.ones(1, dtype=np.float32).astype(ml_dtypes.bfloat16),
        })
    res = run_bass_kernel_spmd(nc, in_maps, core_ids=list(range(NCORES)))
    out = np.concatenate([res.results[c]["y"] for c in range(NCORES)], axis=0)
    return out.astype(np.float32)



# revision 103
# speedup vs baseline: 1.0015x; 1.0001x over previous
"""EvaAttention TRN2 kernel v2: data-parallel over batch across 8 NeuronCores.

Per core (2 batches): bf16 qkv matmuls, joint q/k per-head layernorm stats,
RoPE via folded cos/sin tables, attention with no-max softmax where exp()
tiles are [128,1024] (pair-wide) and PV is computed in flipped orientation
(stationary = probabilities, moving = V) so the output lands as [q, d] in
PSUM -- no transpose and no DRAM round-trip for the attention output.
scale_norm + proj read the SBUF-resident attention output.
"""
import os
import sys

for _p in (
    "/root/.axon_site",
    "/root/.axon_site/_ro/trn_rl_repo",
    "/root/.axon_site/_ro/pypackages",
    "/opt/trn_rl_repo",
    "/opt/pypackages",
):
    if os.path.isdir(_p) and _p not in sys.path:
        sys.path.append(_p)

import numpy as np
import ml_dtypes

import concourse.bass as bass
import concourse.bacc as bacc
import concourse.tile as tile
from concourse import mybir
from concourse.bass_utils import run_bass_kernel_spmd

F32 = mybir.dt.float32
F32R = mybir.dt.float32r
BF16 = mybir.dt.bfloat16
F16 = mybir.dt.float16
Act = mybir.ActivationFunctionType
Alu = mybir.AluOpType
X = mybir.AxisListType.X

B, N, C, H, D = 16, 1025, 1024, 16, 64
EPS = 1e-6
SCALE = D ** -0.5
NCORES = 8
BL = B // NCORES          # batches per core
NT = 9                    # token tiles per batch (pad 1025 -> 1152)
NPAD = NT * 128
HH = 2                    # head halves
HPH = H // HH             # heads per half (8)
PAIRS = HPH // 2          # head pairs per half (4)

_CACHE = {}


def _bcast_mid(ap2d, n):
    """[P, F] AP -> [P, n, F] with step-0 middle dim (free-dim broadcast)."""
    return bass.AP(tensor=ap2d.tensor, offset=ap2d.offset,
                   ap=[ap2d.ap[0], [0, n], ap2d.ap[1]])


def _bcast_last(ap2d, n):
    """[P, F] AP -> [P, F, n] with step-0 last dim."""
    return bass.AP(tensor=ap2d.tensor, offset=ap2d.offset,
                   ap=[ap2d.ap[0], ap2d.ap[1], [0, n]])


class _Pools:
    pass


def _build(has_kbias, has_pbias):
    nc = bacc.Bacc("TRN2", target_bir_lowering=False, debug=False,
                   num_devices=NCORES)

    x_in = nc.dram_tensor("x", [BL, NPAD, C], F16,
                          kind="ExternalInput").ap()
    wt = nc.dram_tensor("wt", [C, 3 * C], F16, kind="ExternalInput").ap()
    nbc = 3 * C
    qkvb = nc.dram_tensor("qkvb", [nbc], F16, kind="ExternalInput").ap()
    ropet = nc.dram_tensor("ropet", [2, NPAD, D], F16,
                           kind="ExternalInput").ap()
    pwt = nc.dram_tensor("pwt", [C, C], F16, kind="ExternalInput").ap()
    wsum = nc.dram_tensor("wsum", [C, 2, 16], F16,
                          kind="ExternalInput").ap()
    bsum = nc.dram_tensor("bsum", [2, 16], F32, kind="ExternalInput").ap()
    pbias = nc.dram_tensor("pbias", [C], F32, kind="ExternalInput").ap()
    identh_d = nc.dram_tensor("identh", [128, 128], F16,
                              kind="ExternalInput").ap()
    identb_d = nc.dram_tensor("identb", [128, 128], BF16,
                              kind="ExternalInput").ap()
    onesb_d = nc.dram_tensor("onesb", [1], BF16, kind="ExternalInput").ap()
    y = nc.dram_tensor("y", [BL, N, C], F32, kind="ExternalOutput").ap()

    with tile.TileContext(nc, pool_alloc_mode="queue") as tc:
        import contextlib
        ctx = contextlib.ExitStack()
        with ctx:
            P = _Pools()
            consts = ctx.enter_context(tc.tile_pool(name="consts", bufs=1))
            P.identh = consts.tile([128, 128], F16, name="identh")
            nc.sync.dma_start(out=P.identh, in_=identh_d)
            P.epst = consts.tile([128, 1], F32, name="epst")
            nc.vector.memset(P.epst, EPS)
            # shared rope tables: [0]=cos [1]=sin (kn_g-folded)
            P.rtab = consts.tile([128, 2, NT, D], F16, name="rtab")
            P.onesh = consts.tile([1, 128], F16, name="onesh")
            nc.vector.memset(P.onesh, 1.0)
            P.biasb = consts.tile([128, nbc], F16, name="biasb")
            nc.sync.dma_start(
                out=P.biasb,
                in_=bass.AP(tensor=qkvb.tensor, offset=qkvb.offset,
                            ap=[[0, 128], [1, nbc]]))
            P.onesb = onesb_d
            if has_pbias:
                P.pbb = consts.tile([128, C], F32, name="pbb")
                nc.sync.dma_start(
                    out=P.pbb,
                    in_=bass.AP(tensor=pbias.tensor, offset=pbias.offset,
                                ap=[[0, 128], [1, C]]))
            else:
                P.pbb = None
            P.wsum = consts.tile([128, 8, 2, 16], F16, name="wsum")
            nc.sync.dma_start(
                out=P.wsum,
                in_=wsum.rearrange("(k p) a s -> p k a s", p=128))
            P.bsum = consts.tile([128, 2, 16], F32, name="bsum")
            nc.sync.dma_start(
                out=P.bsum,
                in_=bass.AP(tensor=bsum.tensor, offset=bsum.offset,
                            ap=[[0, 128], [1, 32]]))
            P.pw = [consts.tile([128, 8, 512], F16, name=f"pwr{oc}")
                    for oc in range(2)]

            # PSUM pools: st 2x2banks, pv 2x1, mm512 3x1  (total 8 banks... 9)
            P.pst = ctx.enter_context(
                tc.tile_pool(name="pst", bufs=2, space="PSUM"))
            P.ppv = ctx.enter_context(
                tc.tile_pool(name="ppv", bufs=1, space="PSUM"))
            P.pmm = ctx.enter_context(
                tc.tile_pool(name="pmm", bufs=2, space="PSUM"))

            # SBUF working pools
            P.xtp = ctx.enter_context(tc.tile_pool(name="xtp", bufs=1))
            P.wp = ctx.enter_context(tc.tile_pool(name="wp", bufs=1))
            P.qkt = ctx.enter_context(tc.tile_pool(name="qkt", bufs=2))
            P.prep = ctx.enter_context(tc.tile_pool(name="prep", bufs=1))
            P.stp = ctx.enter_context(tc.tile_pool(name="stp", bufs=2))
            P.ptp = ctx.enter_context(tc.tile_pool(name="ptp", bufs=10))
            P.attnp = ctx.enter_context(tc.tile_pool(name="attnp", bufs=1))
            P.drp = ctx.enter_context(tc.tile_pool(name="drp", bufs=4))
            # (identb input left declared but unused on-chip)
            P.lnp = ctx.enter_context(tc.tile_pool(name="lnp", bufs=1))
            P.outp = ctx.enter_context(tc.tile_pool(name="outp", bufs=1))

            nc._has_kbias = has_kbias

            # emission order staggers norm_proj(b-1) after qkv(b, h0) so
            # the pmm pool rotation doesn't chain the next batch's qkv
            # behind the previous batch's attention.
            pending = None  # (b, attn) awaiting norm_proj
            for b in range(BL):
                xT = _build_xt(nc, P, b, x_in)
                if b == 0:
                    _late_consts(nc, P, ropet, pwt)
                attn = P.attnp.tile([128, NT, C], F16, tag="attn",
                                    name=f"attn{b}")
                nc.vector.memset(attn[:, NT - 1, :], 0.0)
                for hh in range(HH):
                    QT, KT, V = _qkv_half(nc, P, b, hh, wt, xT)
                    if hh == 0 and pending is not None:
                        _norm_proj(nc, P, pending[0], pending[1], y)
                        pending = None
                    _attn_half(nc, P, b, hh, QT, KT, V, attn)
                pending = (b, attn)
            _norm_proj(nc, P, pending[0], pending[1], y)
    nc.compile()
    return nc


def _late_consts(nc, P, ropet, pwt):
    nc.sync.dma_start(
        out=P.rtab, in_=ropet.rearrange("f (t p) d -> p f t d", p=128))
    for oc in range(2):
        nc.sync.dma_start(
            out=P.pw[oc],
            in_=pwt[:, oc * 512:(oc + 1) * 512].rearrange(
                "(k p) o -> p k o", p=128))


def _build_xt(nc, P, b, x_in):
    """Load x[b]^T via xbar-transpose DMA into xT [128c, 8k, NPAD] bf16."""
    xT = P.xtp.tile([128, 8, NPAD], F16, tag="xT", name=f"xT{b}")
    for k in range(8):
        nc.sync.dma_start_transpose(out=xT[:, k, :],
                                    in_=x_in[b, :, k * 128:(k + 1) * 128])
    return xT


def _qkv_half(nc, P, b, hh, wt, xT):
    """qkv matmuls for one head-half + LN + RoPE -> QT/KT (bf16) and V."""
    QT = P.qkt.tile([128, PAIRS, NPAD], F16, tag="QT", name=f"QT{b}_{hh}")
    KT = P.qkt.tile([128, PAIRS, NPAD], F16, tag="KT", name=f"KT{b}_{hh}")
    V = P.qkt.tile([128, NT, HPH, D + 1], BF16, tag="V", name=f"V{b}_{hh}")
    # ones column of V (col D); untouched pad rows are never read
    nc.sync.dma_start(
        out=V[:, :, :, D:D + 1].rearrange("p t h o -> p (t h) o"),
        in_=bass.AP(tensor=P.onesb.tensor, offset=P.onesb.offset,
                    ap=[[0, 128], [0, NT * HPH], [1, 1]]))
    wch = []
    for oc in range(3):
        col0 = hh * 1536 + oc * 512
        w = P.wp.tile([128, 8, 512], F16, tag=f"w{oc}", name=f"w{oc}")
        nc.sync.dma_start(
            out=w, in_=wt[:, col0:col0 + 512].rearrange("(k p) o -> p k o",
                                                        p=128))
        wch.append(w)

    sgrp = P.prep.tile([128, 2, NT, 512], F16, tag="sgrp",
                       name=f"sgrp{b}{hh}",
                       padded_shape=None)
    svar = P.stp.tile([128, NT, 16], F32, tag="svar", name="svar")
    half = D // 2
    for t in range(NT):
        raws = []
        s2 = P.stp.tile([128, 2, HPH], F32, tag="s2", name="s2")
        psum16 = P.pmm.tile([128, 16], F32, tag="mm512", name="sumps")
        for k in range(8):
            nc.tensor.matmul(psum16, xT[:, k, t * 128:(t + 1) * 128],
                             P.wsum[:, k, hh, :], start=(k == 0),
                             stop=(k == 7))
        sums = P.stp.tile([128, 2, HPH], F32, tag="sums", name="sums")
        nc.vector.tensor_tensor(out=sums.rearrange("p a h -> p (a h)"),
                                in0=psum16, in1=P.bsum[:, hh, :],
                                op=Alu.add)
        # mu first (from the sums matmul) so the evacuation centers in place
        mu = P.stp.tile([128, 2, HPH], F32, tag="mu", name="mu")
        nc.vector.tensor_scalar(mu.rearrange("p a h -> p (a h)"),
                                sums.rearrange("p a h -> p (a h)"),
                                1.0 / D, None, op0=Alu.mult)
        for oc in range(2):
            ps = P.pmm.tile([128, 512], F32, tag="mm512", name="qkps")
            for k in range(8):
                nc.tensor.matmul(ps, xT[:, k, t * 128:(t + 1) * 128],
                                 wch[oc][:, k, :], start=(k == 0),
                                 stop=False)
            bc0 = hh * 1536 + oc * 512
            nc.tensor.matmul(ps, P.onesh[:, 0:128],
                             P.biasb[0:1, bc0:bc0 + 512], start=False,
                             stop=True)
            cen = P.prep.tile([128, HPH, D], F16, tag=f"raw{oc}",
                              name=f"cen{oc}")
            nc.vector.tensor_tensor(
                out=cen, in0=ps.rearrange("p (h d) -> p h d", h=HPH),
                in1=_bcast_last(mu[:, oc, :], D), op=Alu.subtract)
            cenf = cen.rearrange("p h d -> p (h d)")
            sq = P.prep.tile([128, HPH * D], F16, tag=f"rb{oc}", name="sq")
            nc.vector.tensor_tensor(out=sq, in0=cenf, in1=cenf, op=Alu.mult)
            nc.vector.tensor_reduce(
                s2[:, oc, :], sq.rearrange("p (h d) -> p h d", h=HPH),
                axis=X, op=Alu.add)
            raws.append(cen)
        nc.vector.tensor_scalar(
            svar[:, t, :], s2.rearrange("p a h -> p (a h)"), 1.0 / D, None,
            op0=Alu.mult)
        # ---- RoPE on centered values: s = cen*cos + swap(cen)*sin ----
        for oc in range(2):
            ctab = P.rtab[:, 0, t, :]
            stab = P.rtab[:, 1, t, :]
            t1 = raws[oc]
            ra = P.prep.tile([128, HPH, D], F16, tag=f"ra{oc}", name="ra")
            nc.gpsimd.tensor_tensor(out=ra, in0=t1,
                                    in1=_bcast_mid(ctab, HPH), op=Alu.mult)
            rb = P.prep.tile([128, HPH, D], F16, tag=f"rb{oc}", name="rb")
            nc.vector.tensor_tensor(
                out=rb[:, :, 0:half], in0=t1[:, :, half:D],
                in1=_bcast_mid(stab[:, 0:half], HPH), op=Alu.mult)
            nc.vector.tensor_tensor(
                out=rb[:, :, half:D], in0=t1[:, :, 0:half],
                in1=_bcast_mid(stab[:, half:D], HPH), op=Alu.mult)
            nc.gpsimd.tensor_tensor(
                out=sgrp[:, oc, t, :].rearrange("p (h d) -> p h d", h=HPH),
                in0=ra, in1=rb, op=Alu.add)
        # ---- v ----
        ps = P.pmm.tile([128, 512], F32, tag="mm512", name="vps")
        for k in range(8):
            nc.tensor.matmul(ps, xT[:, k, t * 128:(t + 1) * 128],
                             wch[2][:, k, :], start=(k == 0), stop=(k == 7))
        nc.scalar.copy(out=V[:, t, :, 0:D],
                       in_=ps.rearrange("p (h d) -> p h d", h=HPH))
    # ---- rstd in two chunks so pass 2 / attention start earlier ----
    rstd = P.stp.tile([128, NT, 16], F32, tag="rstd", name="rstd")
    for ci, (t0, t1) in enumerate(((0, 5), (5, NT))):
        sd = P.stp.tile([128, (t1 - t0) * 16], F32, tag=f"sd{ci}",
                        name="sd")
        nc.scalar.activation(
            sd, svar[:, t0:t1, :].rearrange("p t s -> p (t s)"), Act.Sqrt,
            bias=P.epst[:, 0:1])
        nc.vector.reciprocal(
            rstd[:, t0:t1, :].rearrange("p t s -> p (t s)"), sd)
        nc.vector.tensor_scalar(rstd[:, t0:t1, 0:HPH],
                                rstd[:, t0:t1, 0:HPH], SCALE,
                                None, op0=Alu.mult)
        _pass2(nc, P, t0, t1, sgrp, rstd, QT, KT)
    return QT, KT, V


def _pass2(nc, P, t0, t1, sgrp, rstd, QT, KT):
    # ---- pass 2: scale by rstd, transpose into QT/KT ----
    for t in range(t0, t1):
        for oc in range(2):
            rot = P.prep.tile([128, HPH, D], F16, tag=f"ra{oc}",
                              name="rot")
            nc.vector.tensor_tensor(
                out=rot,
                in0=sgrp[:, oc, t, :].rearrange("p (h d) -> p h d", h=HPH),
                in1=_bcast_last(rstd[:, t, oc * HPH:(oc + 1) * HPH], D),
                op=Alu.mult)
            rotf = rot.rearrange("p h d -> p (h d)")
            psg = P.pmm.tile([128, 4, 128], F16, tag="mm512", name="qktr")
            for p in range(PAIRS):
                nc.tensor.transpose(psg[:, p, :],
                                    rotf[:, p * 128:(p + 1) * 128],
                                    P.identh[:])
            dst = (QT if oc == 0 else KT)[:, :, t * 128:(t + 1) * 128]
            nc.vector.tensor_copy(out=dst, in_=psg)


def _attn_half(nc, P, b, hh, QT, KT, V, attn):
    """Attention for 8 heads of one half; PV flipped -> attn [q, d] bf16."""
    for qc in range(2):
        q0 = qc * 512
        for pp in range(PAIRS):
            heads = (2 * pp, 2 * pp + 1)
            pv = [P.ppv.tile([128, 2, 2, D + 1], F32, tag=f"pv{jj}",
                             name=f"pv{jj}") for jj in range(2)]
            pts = [None] * 9
            for kt in range(9):
                if kt < 8:
                    st = P.pst.tile([128, 2, 512], F32, tag="st", name="st")
                    for s in range(2):
                        r = 64 * s
                        nc.tensor.matmul(
                            st[:, s, :],
                            KT[r:r + 64, pp, kt * 128:(kt + 1) * 128],
                            QT[r:r + 64, pp, q0:q0 + 512])
                    pt = P.ptp.tile([128, 2, 512], BF16, tag="pt", name="pt")
                    nc.scalar.activation(pt, st, Act.Exp)
                else:
                    # k straggler (token 1024): [1, 512] rows per head
                    st = P.pst.tile([128, 2, 512], F32, tag="st", name="st8")
                    for s in range(2):
                        r = 64 * s
                        nc.tensor.matmul(
                            st[0:1, s, :], KT[r:r + 64, pp, 1024:1025],
                            QT[r:r + 64, pp, q0:q0 + 512])
                    pt = P.ptp.tile([128, 2, 512], BF16, tag="pt", name="pt8")
                    nc.scalar.activation(pt[0:1, :, :], st[0:1, :, :],
                                         Act.Exp)
                pts[kt] = pt
            for s in range(2):
                hl = heads[s]
                for j in range(4):
                    dstpv = pv[j // 2][:, j % 2, s, :]
                    for kt in range(8):
                        nc.tensor.matmul(
                            dstpv, pts[kt][:, s, j * 128:(j + 1) * 128],
                            V[:, kt, hl, :], start=(kt == 0), stop=False)
                    nc.tensor.matmul(
                        dstpv, pts[8][0:1, s, j * 128:(j + 1) * 128],
                        V[0:1, 8, hl, :], start=False, stop=True)
            # drains: per (j, s): reciprocal of denominator, scale, store
            for jj in range(2):
                for j2 in range(2):
                    j = 2 * jj + j2
                    for s in range(2):
                        hg = hh * HPH + heads[s]
                        rl = P.drp.tile([128, 1], F32, tag="rl", name="rl")
                        nc.vector.reciprocal(rl, pv[jj][:, j2, s, D:D + 1])
                        dst = attn[:, qc * 4 + j, hg * D:(hg + 1) * D]
                        vc0 = hh * 1536 + 1024
                        bv = P.biasb[:, vc0 + heads[s] * D:
                                     vc0 + (heads[s] + 1) * D]
                        nc.vector.scalar_tensor_tensor(
                            out=dst, in0=pv[jj][:, j2, s, 0:D],
                            scalar=rl[:, 0:1], in1=bv,
                            op0=Alu.mult, op1=Alu.add)
        # ---- q straggler: token 1024 (partition 0 of tile 8) ----
        if qc == 1:
            for pp in range(PAIRS):
                heads = (2 * pp, 2 * pp + 1)
                sp1 = P.pst.tile([128, 18], F32, tag="st", name="sp1")
                for s in range(2):
                    r = 64 * s
                    qstr = QT[r:r + 64, pp, 1024:1025]
                    for kt in range(8):
                        nc.tensor.matmul(
                            sp1[:, 9 * s + kt:9 * s + kt + 1],
                            KT[r:r + 64, pp, kt * 128:(kt + 1) * 128], qstr)
                    nc.tensor.matmul(sp1[0:1, 9 * s + 8:9 * s + 9],
                                     KT[r:r + 64, pp, 1024:1025], qstr)
                p1 = P.ptp.tile([128, 18], BF16, tag="p1", name="p1")
                nc.scalar.activation(p1, sp1, Act.Exp)
                pv1 = P.ppv.tile([128, 2, D + 1], F32, tag="pv0",
                                 name="pvstr")
                for s in range(2):
                    hl = heads[s]
                    for kt in range(8):
                        nc.tensor.matmul(
                            pv1[0:1, s, :], p1[:, 9 * s + kt:9 * s + kt + 1],
                            V[:, kt, hl, :], start=(kt == 0), stop=False)
                    nc.tensor.matmul(pv1[0:1, s, :],
                                     p1[0:1, 9 * s + 8:9 * s + 9],
                                     V[0:1, 8, hl, :], start=False, stop=True)
                for s in range(2):
                    hg = hh * HPH + heads[s]
                    rl1 = P.drp.tile([128, 1], F32, tag="rl", name="rl1")
                    nc.vector.reciprocal(rl1[0:1, :], pv1[0:1, s, D:D + 1])
                    vc0 = hh * 1536 + 1024
                    bv = P.biasb[0:1, vc0 + heads[s] * D:
                                 vc0 + (heads[s] + 1) * D]
                    nc.vector.scalar_tensor_tensor(
                        out=attn[0:1, 8, hg * D:(hg + 1) * D],
                        in0=pv1[0:1, s, 0:D], scalar=rl1[0:1, 0:1], in1=bv,
                        op0=Alu.mult, op1=Alu.add)


def _norm_proj(nc, P, b, attn, y):
    """scale_norm over C + proj matmul + output DMA for batch b."""
    svn = P.stp.tile([128, NT, 2], F32, tag="svn", name="svn")
    for t in range(NT):
        bnt = P.stp.tile([128, 2, 6], F32, tag="bnt", name="bnt")
        for g in range(2):
            nc.vector.bn_stats(bnt[:, g, :],
                               attn[:, t, g * 512:(g + 1) * 512])
        nc.vector.bn_aggr(svn[:, t, :], bnt.rearrange("p g s -> p (g s)"))
    sdn = P.stp.tile([128, NT], F32, tag="sdn", name="sdn")
    nc.scalar.activation(sdn, svn[:, :, 1], Act.Sqrt, bias=P.epst[:, 0:1])
    rstdn = P.stp.tile([128, NT], F32, tag="rstdn", name="rstdn")
    nc.vector.reciprocal(rstdn, sdn)
    for t in range(NT):
        ln = P.lnp.tile([128, C], F16, tag="ln", name="ln")
        nc.gpsimd.tensor_scalar(ln, attn[:, t, :], svn[:, t, 0:1],
                                rstdn[:, t:t + 1], op0=Alu.subtract,
                                op1=Alu.mult)
        lnT = P.lnp.tile([128, 8, 128], F16, tag="lnT", name="lnT")
        for g in range(2):
            psg = P.pmm.tile([128, 4, 128], F16, tag="mm512", name="lntr")
            for k in range(4):
                nc.tensor.transpose(psg[:, k, :],
                                    ln[:, (4 * g + k) * 128:
                                       (4 * g + k + 1) * 128],
                                    P.identh[:])
            dst = lnT[:, 4 * g:4 * g + 4, :]
            nc.vector.tensor_copy(out=dst, in_=psg)
        ostage = P.outp.tile([128, C], F32, tag="ostage", name="ostage")
        for oc in range(2):
            ps = P.pmm.tile([128, 512], F32, tag="mm512", name="projps")
            for k in range(8):
                nc.tensor.matmul(ps, lnT[:, k, :], P.pw[oc][:, k, :],
                                 start=(k == 0), stop=(k == 7))
            dst = ostage[:, oc * 512:(oc + 1) * 512]
            if P.pbb is not None:
                ee = nc.vector if oc == 0 else nc.gpsimd
                ee.tensor_tensor(out=dst, in0=ps,
                                 in1=P.pbb[:, oc * 512:(oc + 1) * 512],
                                 op=Alu.add)
            else:
                nc.vector.tensor_copy(out=dst, in_=ps)
        rows = 128 if t < NT - 1 else N - 128 * (NT - 1)
        nc.sync.dma_start(out=y[b, t * 128:t * 128 + rows, :],
                          in_=ostage[:rows, :])


def _host_prep(inputs):
    """Precompute permuted/transposed weights and folded rope tables."""
    perm = np.concatenate([np.arange(0, D, 2), np.arange(1, D, 2)])
    swap = np.concatenate([np.arange(D // 2, D), np.arange(0, D // 2)])

    qkv_w = np.asarray(inputs["qkv_w"], np.float32)
    rope = np.asarray(inputs["rope"], np.float32)
    sin_t, cos_t = rope[:, :D], rope[:, D:]

    # column order: [half][q|k|v][head-in-half][d]  (d permuted for q,k)
    row_order = np.empty(3 * C, np.int64)
    col = 0
    for hh in range(HH):
        for grp in range(3):
            for h in range(hh * HPH, (hh + 1) * HPH):
                base = grp * C + h * D
                idx = base + (perm if grp < 2 else np.arange(D))
                row_order[col:col + D] = idx
                col += D
    wt = np.ascontiguousarray(qkv_w[row_order, :].T)  # [C, 3C]

    qb = np.asarray(inputs["q_bias"], np.float32)
    kb = np.asarray(inputs["k_bias"], np.float32)
    vb = np.asarray(inputs["v_bias"], np.float32)
    full_bias = np.concatenate([qb, kb, vb])
    qkvb3 = full_bias[row_order].astype(np.float32)
    qkvb = qkvb3

    def make_tables(g, scale):
        gp = np.asarray(g, np.float32)[perm]          # g in permuted coords
        cos_p = cos_t[:, perm]                        # [1024, D]
        sin_p = sin_t[:, perm]
        sgn = np.where(np.arange(D) < D // 2, -1.0, 1.0).astype(np.float32)
        cost = np.zeros((NPAD, D), np.float32)
        sint = np.zeros((NPAD, D), np.float32)
        cost[0] = gp * scale
        cost[1:N] = cos_p * gp[None, :] * scale
        sint[1:N] = sin_p * sgn[None, :] * gp[swap][None, :] * scale
        return cost, sint

    assert np.allclose(np.asarray(inputs["qn_g"]),
                       np.asarray(inputs["kn_g"])), \
        "kernel specialized for qn_g == kn_g (shared rope tables)"
    ck, sk = make_tables(inputs["kn_g"], 1.0)
    ropet = np.stack([ck, sk])  # [2, NPAD, D]

    # per-head column sums of wt for q,k of each half: [C, 2(hh), 16]
    wsum = np.zeros((C, 2, 16), np.float32)
    bsum = np.zeros((2, 16), np.float32)
    for hh_ in range(HH):
        for oc_ in range(2):
            for h_ in range(HPH):
                cols = slice(hh_ * 1536 + oc_ * 512 + h_ * D,
                             hh_ * 1536 + oc_ * 512 + (h_ + 1) * D)
                wsum[:, hh_, oc_ * HPH + h_] = wt[:, cols].sum(1)
                bsum[hh_, oc_ * HPH + h_] = qkvb3[cols].sum()

    norm_g = np.asarray(inputs["norm_g"], np.float32)
    norm_b = np.asarray(inputs["norm_b"], np.float32)
    proj_w = np.asarray(inputs["proj_w"], np.float32)
    proj_b = np.asarray(inputs["proj_b"], np.float32)
    pwt = np.ascontiguousarray((proj_w * norm_g[None, :]).T)  # [C, C]
    pbias = (proj_b + norm_b @ proj_w.T).astype(np.float32)

    return wt, qkvb, ropet, pwt, pbias, wsum, bsum


def kernel(**inputs):
    qn_b = np.asarray(inputs["qn_b"], np.float32)
    kn_b = np.asarray(inputs["kn_b"], np.float32)
    assert not qn_b.any() and not kn_b.any(), \
        "kernel specialized for qn_b == kn_b == 0"

    (wt, qkvb, ropet, pwt, pbias, wsum,
     bsum) = _host_prep(inputs)
    has_kbias = bool(np.asarray(inputs["k_bias"]).any())
    has_pbias = bool(pbias.any())

    key = (has_kbias, has_pbias)
    if key not in _CACHE:
        _CACHE[key] = _build(has_kbias, has_pbias)
    nc = _CACHE[key]

    x = np.asarray(inputs["x"], np.float32)
    xp = np.zeros((B, NPAD, C), np.float16)
    xp[:, :N] = x.astype(np.float16)
    in_maps = []
    for c in range(NCORES):
        in_maps.append({
            "x": np.ascontiguousarray(xp[c * BL:(c + 1) * BL]),
            "wt": wt.astype(np.float16),
            "qkvb": qkvb.astype(np.float16),
            "ropet": ropet.astype(np.float16),
            "pwt": pwt.astype(np.float16),
            "wsum": wsum.astype(np.float16),
            "bsum": bsum,
            "pbias": pbias,
            "identh": np.eye(128, dtype=np.float16),
            "identb": np.eye(128, dtype=np.float32).astype(ml_dtypes.bfloat16),
            "onesb": np.ones(1, dtype=np.float32).astype(ml_dtypes.bfloat16),
        })
    res = run_bass_kernel_spmd(nc, in_maps, core_ids=list(range(NCORES)))
    out = np.concatenate([res.results[c]["y"] for c in range(NCORES)], axis=0)
    return out.astype(np.float32)



# revision 106
# speedup vs baseline: 1.0103x; 1.0088x over previous
"""EvaAttention TRN2 kernel v2: data-parallel over batch across 8 NeuronCores.

Per core (2 batches): bf16 qkv matmuls, joint q/k per-head layernorm stats,
RoPE via folded cos/sin tables, attention with no-max softmax where exp()
tiles are [128,1024] (pair-wide) and PV is computed in flipped orientation
(stationary = probabilities, moving = V) so the output lands as [q, d] in
PSUM -- no transpose and no DRAM round-trip for the attention output.
scale_norm + proj read the SBUF-resident attention output.
"""
import os
import sys

for _p in (
    "/root/.axon_site",
    "/root/.axon_site/_ro/trn_rl_repo",
    "/root/.axon_site/_ro/pypackages",
    "/opt/trn_rl_repo",
    "/opt/pypackages",
):
    if os.path.isdir(_p) and _p not in sys.path:
        sys.path.append(_p)

import numpy as np
import ml_dtypes

import concourse.bass as bass
import concourse.bacc as bacc
import concourse.tile as tile
from concourse import mybir
from concourse.bass_utils import run_bass_kernel_spmd

F32 = mybir.dt.float32
F32R = mybir.dt.float32r
BF16 = mybir.dt.bfloat16
F16 = mybir.dt.float16
Act = mybir.ActivationFunctionType
Alu = mybir.AluOpType
X = mybir.AxisListType.X

B, N, C, H, D = 16, 1025, 1024, 16, 64
EPS = 1e-6
SCALE = D ** -0.5
NCORES = 8
BL = B // NCORES          # batches per core
NT = 9                    # token tiles per batch (pad 1025 -> 1152)
NPAD = NT * 128
HH = 2                    # head halves
HPH = H // HH             # heads per half (8)
PAIRS = HPH // 2          # head pairs per half (4)

_CACHE = {}


def _bcast_mid(ap2d, n):
    """[P, F] AP -> [P, n, F] with step-0 middle dim (free-dim broadcast)."""
    return bass.AP(tensor=ap2d.tensor, offset=ap2d.offset,
                   ap=[ap2d.ap[0], [0, n], ap2d.ap[1]])


def _bcast_last(ap2d, n):
    """[P, F] AP -> [P, F, n] with step-0 last dim."""
    return bass.AP(tensor=ap2d.tensor, offset=ap2d.offset,
                   ap=[ap2d.ap[0], ap2d.ap[1], [0, n]])


class _Pools:
    pass


def _build(has_kbias, has_pbias):
    nc = bacc.Bacc("TRN2", target_bir_lowering=False, debug=False,
                   num_devices=NCORES)

    x_in = nc.dram_tensor("x", [BL, NPAD, C], F16,
                          kind="ExternalInput").ap()
    wt = nc.dram_tensor("wt", [C, 3 * C], F16, kind="ExternalInput").ap()
    nbc = 3 * C
    qkvb = nc.dram_tensor("qkvb", [nbc], F16, kind="ExternalInput").ap()
    ropet = nc.dram_tensor("ropet", [2, NPAD, D], F16,
                           kind="ExternalInput").ap()
    pwt = nc.dram_tensor("pwt", [C, C], F16, kind="ExternalInput").ap()
    wsum = nc.dram_tensor("wsum", [C, 2, 16], F16,
                          kind="ExternalInput").ap()
    bsum = nc.dram_tensor("bsum", [2, 16], F32, kind="ExternalInput").ap()
    pbias = nc.dram_tensor("pbias", [C], F32, kind="ExternalInput").ap()
    identh_d = nc.dram_tensor("identh", [128, 128], F16,
                              kind="ExternalInput").ap()
    identb_d = nc.dram_tensor("identb", [128, 128], BF16,
                              kind="ExternalInput").ap()
    onesb_d = nc.dram_tensor("onesb", [1], BF16, kind="ExternalInput").ap()
    y = nc.dram_tensor("y", [BL, N, C], F32, kind="ExternalOutput").ap()

    with tile.TileContext(nc, pool_alloc_mode="queue") as tc:
        import contextlib
        ctx = contextlib.ExitStack()
        with ctx:
            P = _Pools()
            consts = ctx.enter_context(tc.tile_pool(name="consts", bufs=1))
            P.identh = consts.tile([128, 128], F16, name="identh")
            nc.sync.dma_start(out=P.identh, in_=identh_d)
            P.epst = consts.tile([128, 1], F32, name="epst")
            nc.vector.memset(P.epst, EPS)
            # shared rope tables: [0]=cos [1]=sin (kn_g-folded)
            P.rtab = consts.tile([128, 2, NT, D], F16, name="rtab")
            P.onesh = consts.tile([1, 128], F16, name="onesh")
            nc.vector.memset(P.onesh, 1.0)
            P.biasb = consts.tile([128, nbc], F16, name="biasb")
            nc.sync.dma_start(
                out=P.biasb,
                in_=bass.AP(tensor=qkvb.tensor, offset=qkvb.offset,
                            ap=[[0, 128], [1, nbc]]))
            P.onesb = onesb_d
            if has_pbias:
                P.pbb = consts.tile([128, C], F32, name="pbb")
                nc.sync.dma_start(
                    out=P.pbb,
                    in_=bass.AP(tensor=pbias.tensor, offset=pbias.offset,
                                ap=[[0, 128], [1, C]]))
            else:
                P.pbb = None
            P.wsum = consts.tile([128, 8, 2, 16], F16, name="wsum")
            nc.sync.dma_start(
                out=P.wsum,
                in_=wsum.rearrange("(k p) a s -> p k a s", p=128))
            P.bsum = consts.tile([128, 2, 16], F32, name="bsum")
            nc.sync.dma_start(
                out=P.bsum,
                in_=bass.AP(tensor=bsum.tensor, offset=bsum.offset,
                            ap=[[0, 128], [1, 32]]))
            P.pw = [consts.tile([128, 8, 512], F16, name=f"pwr{oc}")
                    for oc in range(2)]

            # PSUM pools: st 2x2banks, pv 2x1, mm512 3x1  (total 8 banks... 9)
            P.pst = ctx.enter_context(
                tc.tile_pool(name="pst", bufs=2, space="PSUM"))
            P.ppv = ctx.enter_context(
                tc.tile_pool(name="ppv", bufs=1, space="PSUM"))
            P.pmm = ctx.enter_context(
                tc.tile_pool(name="pmm", bufs=2, space="PSUM"))

            # SBUF working pools
            P.xtp = ctx.enter_context(tc.tile_pool(name="xtp", bufs=1))
            P.wp = ctx.enter_context(tc.tile_pool(name="wp", bufs=1))
            P.qkt = ctx.enter_context(tc.tile_pool(name="qkt", bufs=2))
            P.prep = ctx.enter_context(tc.tile_pool(name="prep", bufs=1))
            P.stp = ctx.enter_context(tc.tile_pool(name="stp", bufs=2))
            P.ptp = ctx.enter_context(tc.tile_pool(name="ptp", bufs=10))
            P.attnp = ctx.enter_context(tc.tile_pool(name="attnp", bufs=1))
            P.drp = ctx.enter_context(tc.tile_pool(name="drp", bufs=4))
            # (identb input left declared but unused on-chip)
            P.lnp = ctx.enter_context(tc.tile_pool(name="lnp", bufs=1))
            P.outp = ctx.enter_context(tc.tile_pool(name="outp", bufs=1))

            nc._has_kbias = has_kbias

            # emission order staggers norm_proj(b-1) after qkv(b, h0) so
            # the pmm pool rotation doesn't chain the next batch's qkv
            # behind the previous batch's attention.
            pending = None  # (b, attn) awaiting norm_proj
            for b in range(BL):
                xT = _build_xt(nc, P, b, x_in)
                if b == 0:
                    _late_consts(nc, P, ropet, pwt)
                attn = P.attnp.tile([128, NT, C], F16, tag="attn",
                                    name=f"attn{b}")
                nc.vector.memset(attn[:, NT - 1, :], 0.0)
                for hh in range(HH):
                    QT, KT, V = _qkv_half(nc, P, b, hh, wt, xT)
                    if hh == 0 and pending is not None:
                        _norm_proj(nc, P, pending[0], pending[1], y)
                        pending = None
                    _attn_half(nc, P, b, hh, QT, KT, V, attn)
                pending = (b, attn)
            _norm_proj(nc, P, pending[0], pending[1], y)
    nc.compile()
    return nc


def _late_consts(nc, P, ropet, pwt):
    nc.sync.dma_start(
        out=P.rtab, in_=ropet.rearrange("f (t p) d -> p f t d", p=128))
    P._pwt_d = pwt
    P._pw_loaded = False


def _build_xt(nc, P, b, x_in):
    """Load x[b]^T via xbar-transpose DMA into xT [128c, 8k, NPAD] bf16."""
    xT = P.xtp.tile([128, 8, NPAD], F16, tag="xT", name=f"xT{b}")
    for k in range(8):
        nc.sync.dma_start_transpose(out=xT[:, k, :],
                                    in_=x_in[b, :, k * 128:(k + 1) * 128])
    return xT


def _qkv_half(nc, P, b, hh, wt, xT):
    """qkv matmuls for one head-half + LN + RoPE -> QT/KT (bf16) and V."""
    QT = P.qkt.tile([128, PAIRS, NPAD], F16, tag="QT", name=f"QT{b}_{hh}")
    KT = P.qkt.tile([128, PAIRS, NPAD], F16, tag="KT", name=f"KT{b}_{hh}")
    V = P.qkt.tile([128, NT, HPH, D + 1], BF16, tag="V", name=f"V{b}_{hh}")
    # ones column of V (col D); untouched pad rows are never read
    nc.sync.dma_start(
        out=V[:, :, :, D:D + 1].rearrange("p t h o -> p (t h) o"),
        in_=bass.AP(tensor=P.onesb.tensor, offset=P.onesb.offset,
                    ap=[[0, 128], [0, NT * HPH], [1, 1]]))
    wch = []
    for oc in range(3):
        col0 = hh * 1536 + oc * 512
        w = P.wp.tile([128, 8, 512], F16, tag=f"w{oc}", name=f"w{oc}")
        nc.sync.dma_start(
            out=w, in_=wt[:, col0:col0 + 512].rearrange("(k p) o -> p k o",
                                                        p=128))
        wch.append(w)

    sgrp = P.prep.tile([128, 2, NT, 512], F16, tag="sgrp",
                       name=f"sgrp{b}{hh}",
                       padded_shape=None)
    svar = P.stp.tile([128, NT, 16], F32, tag="svar", name="svar")
    half = D // 2
    for t in range(NT):
        raws = []
        s2 = P.stp.tile([128, 2, HPH], F32, tag="s2", name="s2")
        psum16 = P.pmm.tile([128, 16], F32, tag="mm512", name="sumps")
        for k in range(8):
            nc.tensor.matmul(psum16, xT[:, k, t * 128:(t + 1) * 128],
                             P.wsum[:, k, hh, :], start=(k == 0),
                             stop=(k == 7))
        sums = P.stp.tile([128, 2, HPH], F32, tag="sums", name="sums")
        nc.vector.tensor_tensor(out=sums.rearrange("p a h -> p (a h)"),
                                in0=psum16, in1=P.bsum[:, hh, :],
                                op=Alu.add)
        # mu first (from the sums matmul) so the evacuation centers in place
        mu = P.stp.tile([128, 2, HPH], F32, tag="mu", name="mu")
        nc.vector.tensor_scalar(mu.rearrange("p a h -> p (a h)"),
                                sums.rearrange("p a h -> p (a h)"),
                                1.0 / D, None, op0=Alu.mult)
        for oc in range(2):
            ps = P.pmm.tile([128, 512], F32, tag="mm512", name="qkps")
            for k in range(8):
                nc.tensor.matmul(ps, xT[:, k, t * 128:(t + 1) * 128],
                                 wch[oc][:, k, :], start=(k == 0),
                                 stop=False)
            bc0 = hh * 1536 + oc * 512
            nc.tensor.matmul(ps, P.onesh[:, 0:128],
                             P.biasb[0:1, bc0:bc0 + 512], start=False,
                             stop=True)
            cen = P.prep.tile([128, HPH, D], F16, tag=f"raw{oc}",
                              name=f"cen{oc}")
            nc.vector.tensor_tensor(
                out=cen, in0=ps.rearrange("p (h d) -> p h d", h=HPH),
                in1=_bcast_last(mu[:, oc, :], D), op=Alu.subtract)
            cenf = cen.rearrange("p h d -> p (h d)")
            sq = P.prep.tile([128, HPH * D], F16, tag=f"rb{oc}", name="sq")
            nc.vector.tensor_tensor(out=sq, in0=cenf, in1=cenf, op=Alu.mult)
            nc.vector.tensor_reduce(
                s2[:, oc, :], sq.rearrange("p (h d) -> p h d", h=HPH),
                axis=X, op=Alu.add)
            raws.append(cen)
        nc.vector.tensor_scalar(
            svar[:, t, :], s2.rearrange("p a h -> p (a h)"), 1.0 / D, None,
            op0=Alu.mult)
        # ---- RoPE on centered values: s = cen*cos + swap(cen)*sin ----
        for oc in range(2):
            ctab = P.rtab[:, 0, t, :]
            stab = P.rtab[:, 1, t, :]
            t1 = raws[oc]
            ra = P.prep.tile([128, HPH, D], F16, tag=f"ra{oc}", name="ra")
            nc.gpsimd.tensor_tensor(out=ra, in0=t1,
                                    in1=_bcast_mid(ctab, HPH), op=Alu.mult)
            rb = P.prep.tile([128, HPH, D], F16, tag=f"rb{oc}", name="rb")
            nc.vector.tensor_tensor(
                out=rb[:, :, 0:half], in0=t1[:, :, half:D],
                in1=_bcast_mid(stab[:, 0:half], HPH), op=Alu.mult)
            nc.vector.tensor_tensor(
                out=rb[:, :, half:D], in0=t1[:, :, 0:half],
                in1=_bcast_mid(stab[:, half:D], HPH), op=Alu.mult)
            nc.gpsimd.tensor_tensor(
                out=sgrp[:, oc, t, :].rearrange("p (h d) -> p h d", h=HPH),
                in0=ra, in1=rb, op=Alu.add)
        # ---- v ----
        ps = P.pmm.tile([128, 512], F32, tag="mm512", name="vps")
        for k in range(8):
            nc.tensor.matmul(ps, xT[:, k, t * 128:(t + 1) * 128],
                             wch[2][:, k, :], start=(k == 0), stop=(k == 7))
        nc.scalar.copy(out=V[:, t, :, 0:D],
                       in_=ps.rearrange("p (h d) -> p h d", h=HPH))
    # ---- rstd in two chunks so pass 2 / attention start earlier ----
    rstd = P.stp.tile([128, NT, 16], F32, tag="rstd", name="rstd")
    for ci, (t0, t1) in enumerate(((0, 5), (5, NT))):
        sd = P.stp.tile([128, (t1 - t0) * 16], F32, tag=f"sd{ci}",
                        name="sd")
        nc.scalar.activation(
            sd, svar[:, t0:t1, :].rearrange("p t s -> p (t s)"), Act.Sqrt,
            bias=P.epst[:, 0:1])
        nc.vector.reciprocal(
            rstd[:, t0:t1, :].rearrange("p t s -> p (t s)"), sd)
        nc.vector.tensor_scalar(rstd[:, t0:t1, 0:HPH],
                                rstd[:, t0:t1, 0:HPH], SCALE,
                                None, op0=Alu.mult)
        _pass2(nc, P, t0, t1, sgrp, rstd, QT, KT)
    return QT, KT, V


def _pass2(nc, P, t0, t1, sgrp, rstd, QT, KT):
    # ---- pass 2: scale by rstd, transpose into QT/KT ----
    for t in range(t0, t1):
        for oc in range(2):
            rot = P.prep.tile([128, HPH, D], F16, tag=f"ra{oc}",
                              name="rot")
            nc.vector.tensor_tensor(
                out=rot,
                in0=sgrp[:, oc, t, :].rearrange("p (h d) -> p h d", h=HPH),
                in1=_bcast_last(rstd[:, t, oc * HPH:(oc + 1) * HPH], D),
                op=Alu.mult)
            rotf = rot.rearrange("p h d -> p (h d)")
            psg = P.pmm.tile([128, 4, 128], F16, tag="mm512", name="qktr")
            for p in range(PAIRS):
                nc.tensor.transpose(psg[:, p, :],
                                    rotf[:, p * 128:(p + 1) * 128],
                                    P.identh[:])
            dst = (QT if oc == 0 else KT)[:, :, t * 128:(t + 1) * 128]
            nc.vector.tensor_copy(out=dst, in_=psg)


def _attn_half(nc, P, b, hh, QT, KT, V, attn):
    """Attention for 8 heads of one half; PV flipped -> attn [q, d] bf16."""
    for qc in range(2):
        q0 = qc * 512
        for pp in range(PAIRS):
            heads = (2 * pp, 2 * pp + 1)
            pv = [P.ppv.tile([128, 2, 2, D + 1], F32, tag=f"pv{jj}",
                             name=f"pv{jj}") for jj in range(2)]
            pts = [None] * 9
            for kt in range(9):
                if kt < 8:
                    st = P.pst.tile([128, 2, 512], F32, tag="st", name="st")
                    for s in range(2):
                        r = 64 * s
                        nc.tensor.matmul(
                            st[:, s, :],
                            KT[r:r + 64, pp, kt * 128:(kt + 1) * 128],
                            QT[r:r + 64, pp, q0:q0 + 512])
                    pt = P.ptp.tile([128, 2, 512], BF16, tag="pt", name="pt")
                    nc.scalar.activation(pt, st, Act.Exp)
                else:
                    # k straggler (token 1024): [1, 512] rows per head
                    st = P.pst.tile([128, 2, 512], F32, tag="st", name="st8")
                    for s in range(2):
                        r = 64 * s
                        nc.tensor.matmul(
                            st[0:1, s, :], KT[r:r + 64, pp, 1024:1025],
                            QT[r:r + 64, pp, q0:q0 + 512])
                    pt = P.ptp.tile([128, 2, 512], BF16, tag="pt", name="pt8")
                    nc.scalar.activation(pt[0:1, :, :], st[0:1, :, :],
                                         Act.Exp)
                pts[kt] = pt
            for s in range(2):
                hl = heads[s]
                for j in range(4):
                    dstpv = pv[j // 2][:, j % 2, s, :]
                    for kt in range(8):
                        nc.tensor.matmul(
                            dstpv, pts[kt][:, s, j * 128:(j + 1) * 128],
                            V[:, kt, hl, :], start=(kt == 0), stop=False)
                    nc.tensor.matmul(
                        dstpv, pts[8][0:1, s, j * 128:(j + 1) * 128],
                        V[0:1, 8, hl, :], start=False, stop=True)
            # drains: per (j, s): reciprocal of denominator, scale, store
            for jj in range(2):
                for j2 in range(2):
                    j = 2 * jj + j2
                    for s in range(2):
                        hg = hh * HPH + heads[s]
                        rl = P.drp.tile([128, 1], F32, tag="rl", name="rl")
                        nc.vector.reciprocal(rl, pv[jj][:, j2, s, D:D + 1])
                        dst = attn[:, qc * 4 + j, hg * D:(hg + 1) * D]
                        vc0 = hh * 1536 + 1024
                        bv = P.biasb[:, vc0 + heads[s] * D:
                                     vc0 + (heads[s] + 1) * D]
                        nc.vector.scalar_tensor_tensor(
                            out=dst, in0=pv[jj][:, j2, s, 0:D],
                            scalar=rl[:, 0:1], in1=bv,
                            op0=Alu.mult, op1=Alu.add)
        # ---- q straggler: token 1024 (partition 0 of tile 8) ----
        if qc == 1:
            for pp in range(PAIRS):
                heads = (2 * pp, 2 * pp + 1)
                sp1 = P.pst.tile([128, 18], F32, tag="st", name="sp1")
                for s in range(2):
                    r = 64 * s
                    qstr = QT[r:r + 64, pp, 1024:1025]
                    for kt in range(8):
                        nc.tensor.matmul(
                            sp1[:, 9 * s + kt:9 * s + kt + 1],
                            KT[r:r + 64, pp, kt * 128:(kt + 1) * 128], qstr)
                    nc.tensor.matmul(sp1[0:1, 9 * s + 8:9 * s + 9],
                                     KT[r:r + 64, pp, 1024:1025], qstr)
                p1 = P.ptp.tile([128, 18], BF16, tag="p1", name="p1")
                nc.scalar.activation(p1, sp1, Act.Exp)
                pv1 = P.ppv.tile([128, 2, D + 1], F32, tag="pv0",
                                 name="pvstr")
                for s in range(2):
                    hl = heads[s]
                    for kt in range(8):
                        nc.tensor.matmul(
                            pv1[0:1, s, :], p1[:, 9 * s + kt:9 * s + kt + 1],
                            V[:, kt, hl, :], start=(kt == 0), stop=False)
                    nc.tensor.matmul(pv1[0:1, s, :],
                                     p1[0:1, 9 * s + 8:9 * s + 9],
                                     V[0:1, 8, hl, :], start=False, stop=True)
                for s in range(2):
                    hg = hh * HPH + heads[s]
                    rl1 = P.drp.tile([128, 1], F32, tag="rl", name="rl1")
                    nc.vector.reciprocal(rl1[0:1, :], pv1[0:1, s, D:D + 1])
                    vc0 = hh * 1536 + 1024
                    bv = P.biasb[0:1, vc0 + heads[s] * D:
                                 vc0 + (heads[s] + 1) * D]
                    nc.vector.scalar_tensor_tensor(
                        out=attn[0:1, 8, hg * D:(hg + 1) * D],
                        in0=pv1[0:1, s, 0:D], scalar=rl1[0:1, 0:1], in1=bv,
                        op0=Alu.mult, op1=Alu.add)


def _norm_proj(nc, P, b, attn, y):
    """scale_norm over C + proj matmul + output DMA for batch b."""
    if not P._pw_loaded:
        P._pw_loaded = True
        for oc in range(2):
            nc.sync.dma_start(
                out=P.pw[oc],
                in_=P._pwt_d[:, oc * 512:(oc + 1) * 512].rearrange(
                    "(k p) o -> p k o", p=128))
    svn = P.stp.tile([128, NT, 2], F32, tag="svn", name="svn")
    for t in range(NT):
        bnt = P.stp.tile([128, 2, 6], F32, tag="bnt", name="bnt")
        for g in range(2):
            nc.vector.bn_stats(bnt[:, g, :],
                               attn[:, t, g * 512:(g + 1) * 512])
        nc.vector.bn_aggr(svn[:, t, :], bnt.rearrange("p g s -> p (g s)"))
    sdn = P.stp.tile([128, NT], F32, tag="sdn", name="sdn")
    nc.scalar.activation(sdn, svn[:, :, 1], Act.Sqrt, bias=P.epst[:, 0:1])
    rstdn = P.stp.tile([128, NT], F32, tag="rstdn", name="rstdn")
    nc.vector.reciprocal(rstdn, sdn)
    for t in range(NT):
        ln = P.lnp.tile([128, C], F16, tag="ln", name="ln")
        nc.gpsimd.tensor_scalar(ln, attn[:, t, :], svn[:, t, 0:1],
                                rstdn[:, t:t + 1], op0=Alu.subtract,
                                op1=Alu.mult)
        lnT = P.lnp.tile([128, 8, 128], F16, tag="lnT", name="lnT")
        for g in range(2):
            psg = P.pmm.tile([128, 4, 128], F16, tag="mm512", name="lntr")
            for k in range(4):
                nc.tensor.transpose(psg[:, k, :],
                                    ln[:, (4 * g + k) * 128:
                                       (4 * g + k + 1) * 128],
                                    P.identh[:])
            dst = lnT[:, 4 * g:4 * g + 4, :]
            nc.vector.tensor_copy(out=dst, in_=psg)
        ostage = P.outp.tile([128, C], F32, tag="ostage", name="ostage")
        for oc in range(2):
            ps = P.pmm.tile([128, 512], F32, tag="mm512", name="projps")
            for k in range(8):
                nc.tensor.matmul(ps, lnT[:, k, :], P.pw[oc][:, k, :],
                                 start=(k == 0), stop=(k == 7))
            dst = ostage[:, oc * 512:(oc + 1) * 512]
            if P.pbb is not None:
                ee = nc.vector if oc == 0 else nc.gpsimd
                ee.tensor_tensor(out=dst, in0=ps,
                                 in1=P.pbb[:, oc * 512:(oc + 1) * 512],
                                 op=Alu.add)
            else:
                nc.vector.tensor_copy(out=dst, in_=ps)
        rows = 128 if t < NT - 1 else N - 128 * (NT - 1)
        nc.sync.dma_start(out=y[b, t * 128:t * 128 + rows, :],
                          in_=ostage[:rows, :])


def _host_prep(inputs):
    """Precompute permuted/transposed weights and folded rope tables."""
    perm = np.concatenate([np.arange(0, D, 2), np.arange(1, D, 2)])
    swap = np.concatenate([np.arange(D // 2, D), np.arange(0, D // 2)])

    qkv_w = np.asarray(inputs["qkv_w"], np.float32)
    rope = np.asarray(inputs["rope"], np.float32)
    sin_t, cos_t = rope[:, :D], rope[:, D:]

    # column order: [half][q|k|v][head-in-half][d]  (d permuted for q,k)
    row_order = np.empty(3 * C, np.int64)
    col = 0
    for hh in range(HH):
        for grp in range(3):
            for h in range(hh * HPH, (hh + 1) * HPH):
                base = grp * C + h * D
                idx = base + (perm if grp < 2 else np.arange(D))
                row_order[col:col + D] = idx
                col += D
    wt = np.ascontiguousarray(qkv_w[row_order, :].T)  # [C, 3C]

    qb = np.asarray(inputs["q_bias"], np.float32)
    kb = np.asarray(inputs["k_bias"], np.float32)
    vb = np.asarray(inputs["v_bias"], np.float32)
    full_bias = np.concatenate([qb, kb, vb])
    qkvb3 = full_bias[row_order].astype(np.float32)
    qkvb = qkvb3

    def make_tables(g, scale):
        gp = np.asarray(g, np.float32)[perm]          # g in permuted coords
        cos_p = cos_t[:, perm]                        # [1024, D]
        sin_p = sin_t[:, perm]
        sgn = np.where(np.arange(D) < D // 2, -1.0, 1.0).astype(np.float32)
        cost = np.zeros((NPAD, D), np.float32)
        sint = np.zeros((NPAD, D), np.float32)
        cost[0] = gp * scale
        cost[1:N] = cos_p * gp[None, :] * scale
        sint[1:N] = sin_p * sgn[None, :] * gp[swap][None, :] * scale
        return cost, sint

    assert np.allclose(np.asarray(inputs["qn_g"]),
                       np.asarray(inputs["kn_g"])), \
        "kernel specialized for qn_g == kn_g (shared rope tables)"
    ck, sk = make_tables(inputs["kn_g"], 1.0)
    ropet = np.stack([ck, sk])  # [2, NPAD, D]

    # per-head column sums of wt for q,k of each half: [C, 2(hh), 16]
    wsum = np.zeros((C, 2, 16), np.float32)
    bsum = np.zeros((2, 16), np.float32)
    for hh_ in range(HH):
        for oc_ in range(2):
            for h_ in range(HPH):
                cols = slice(hh_ * 1536 + oc_ * 512 + h_ * D,
                             hh_ * 1536 + oc_ * 512 + (h_ + 1) * D)
                wsum[:, hh_, oc_ * HPH + h_] = wt[:, cols].sum(1)
                bsum[hh_, oc_ * HPH + h_] = qkvb3[cols].sum()

    norm_g = np.asarray(inputs["norm_g"], np.float32)
    norm_b = np.asarray(inputs["norm_b"], np.float32)
    proj_w = np.asarray(inputs["proj_w"], np.float32)
    proj_b = np.asarray(inputs["proj_b"], np.float32)
    pwt = np.ascontiguousarray((proj_w * norm_g[None, :]).T)  # [C, C]
    pbias = (proj_b + norm_b @ proj_w.T).astype(np.float32)

    return wt, qkvb, ropet, pwt, pbias, wsum, bsum


def kernel(**inputs):
    qn_b = np.asarray(inputs["qn_b"], np.float32)
    kn_b = np.asarray(inputs["kn_b"], np.float32)
    assert not qn_b.any() and not kn_b.any(), \
        "kernel specialized for qn_b == kn_b == 0"

    (wt, qkvb, ropet, pwt, pbias, wsum,
     bsum) = _host_prep(inputs)
    has_kbias = bool(np.asarray(inputs["k_bias"]).any())
    has_pbias = bool(pbias.any())

    key = (has_kbias, has_pbias)
    if key not in _CACHE:
        _CACHE[key] = _build(has_kbias, has_pbias)
    nc = _CACHE[key]

    x = np.asarray(inputs["x"], np.float32)
    xp = np.zeros((B, NPAD, C), np.float16)
    xp[:, :N] = x.astype(np.float16)
    in_maps = []
    for c in range(NCORES):
        in_maps.append({
            "x": np.ascontiguousarray(xp[c * BL:(c + 1) * BL]),
            "wt": wt.astype(np.float16),
            "qkvb": qkvb.astype(np.float16),
            "ropet": ropet.astype(np.float16),
            "pwt": pwt.astype(np.float16),
            "wsum": wsum.astype(np.float16),
            "bsum": bsum,
            "pbias": pbias,
            "identh": np.eye(128, dtype=np.float16),
            "identb": np.eye(128, dtype=np.float32).astype(ml_dtypes.bfloat16),
            "onesb": np.ones(1, dtype=np.float32).astype(ml_dtypes.bfloat16),
        })
    res = run_bass_kernel_spmd(nc, in_maps, core_ids=list(range(NCORES)))
    out = np.concatenate([res.results[c]["y"] for c in range(NCORES)], axis=0)
    return out.astype(np.float32)

